# revision 7
# baseline (speedup 1.0000x reference)
"""nn_Intra_ResNet on 8 TRN2 NeuronCores (Bass/Tile, SPMD).

Row-sharded 8-way (48 rows/core) with 20-row halo recompute (no halo
exchange). Activations live in SBUF phase-packed: partition = ch + 64*(row
parity), column = block*392 + 4 + j  (a "block" is a pair of image rows,
392 = 384 + 2*4 zero pad columns). Every 3x3 dilated conv tap is then a
K=128/M=128 fp16 matmul at a column offset (even-d taps and d=1 center
row), or a pair of concurrent 64x64 quadrant matmuls via tile_position
(d=1 phase-flip taps). InstanceNorm: per-partition bn_stats + one [128,2]
fp32 AllReduce per norm (13 total), then a fused scale/bias (+mask) +
LeakyReLU applied with 3 big strided ops (gpsimd z-pass, scalar Lrelu,
vector residual-add).
"""
import sys

for _p in ("/opt/trn_rl_repo",):
    if _p not in sys.path:
        sys.path.insert(0, _p)

import numpy as np

import concourse.bass as bass
import concourse.tile as tile
from concourse import bacc, mybir
from concourse.bass_utils import run_bass_kernel_spmd

f16 = mybir.dt.float16
f32 = mybir.dt.float32
AF = mybir.ActivationFunctionType
OP = mybir.AluOpType

NCOR = 8
L = 384
CH = 64
D1 = 788
D2 = 210
EPS = 1e-5
DILS = [1, 1, 2, 2, 4, 4, 2, 2, 1, 1]
OWN = 48          # rows per core
NBLK = 48         # buffer blocks (96 rows incl 2 guard blocks/side)
WPAD = 392        # padded row width (4 | 384 | 4)
PADL = 4
RG = [list(range(NCOR))]

# halo rows needed before conv q; conv q computes local rows [24-H[q+1], 72+H[q+1])
H = [sum(DILS[i:]) for i in range(11)]          # [20,19,18,16,14,10,6,4,2,1,0]
B0 = [(24 - H[q + 1]) // 2 for q in range(10)]  # [2,3,4,5,7,9,10,11,11,12]
B1 = [(72 + H[q + 1] + 1) // 2 for q in range(10)]
PB0, PB1 = 2, 46   # pair-stage computed blocks
OB0, OB1 = 12, 36  # owned blocks


def _padview(buf, b0, b1):
    """[128, b1-b0, 384] view of padded-layout buffer (data cols only)."""
    base = buf[:, 0:1]
    return bass.AP(tensor=base.tensor, offset=base.offset + b0 * WPAD + PADL,
                   ap=[base.ap[0], [WPAD, b1 - b0], [1, 384]])


def _bcast(t2d, b0, b1):
    """[128, b1-b0, 384] broadcast view of a [128, NBLK] per-block tile."""
    base = t2d[:, b0:b1]
    return bass.AP(tensor=base.tensor, offset=base.offset,
                   ap=[base.ap[0], base.ap[1], [0, 384]])


def _rawview(raw, b0, b1):
    return raw[:, b0 * 384:b1 * 384].rearrange("p (b l) -> p b l", l=384)


def build():
    nc = bacc.Bacc("TRN2", target_bir_lowering=False, debug=False,
                   num_devices=NCOR)
    ein = dict(kind="ExternalInput")
    x1_e = nc.dram_tensor("x1", [896, L], f16, **ein)
    x1l_e = nc.dram_tensor("x1loc", [896, 96], f16, **ein)
    x2_e = nc.dram_tensor("x2", [D2, 96, L], f16, **ein)
    mask_e = nc.dram_tensor("mask", [128, NBLK], f32, **ein)
    w1a_e = nc.dram_tensor("w1a", [7, 128, 64], f16, **ein)
    w1c_e = nc.dram_tensor("w1c", [7, 128, 128], f16, **ein)
    w2a_e = nc.dram_tensor("w2a", [128, 64], f16, **ein)
    w2b_e = nc.dram_tensor("w2b", [128, 64], f16, **ein)
    w3_e = nc.dram_tensor("w3", [2, 128, 128], f16, **ein)
    wf_e = nc.dram_tensor("wf", [10, 9, 128, 128], f16, **ein)
    wh_e = nc.dram_tensor("wh", [10, 6, 128, 64], f16, **ein)
    eyeS_e = nc.dram_tensor("eyeS", [128, 64], f32, **ein)
    eyeD_e = nc.dram_tensor("eyeD", [64, 128], f32, **ein)
    gp_e = nc.dram_tensor("gpair", [64, 3], f32, **ein)
    bp_e = nc.dram_tensor("bpair", [64, 3], f32, **ein)
    gr_e = nc.dram_tensor("gres", [64, 10], f32, **ein)
    br_e = nc.dram_tensor("bres", [64, 10], f32, **ein)
    out_e = nc.dram_tensor("out", [CH, OWN, L], f32, kind="ExternalOutput")

    with tile.TileContext(nc) as tc:
        _body(nc, tc, locals())
    nc.compile()
    return nc


def _body(nc, tc, ten):
    from contextlib import ExitStack
    ctx = ExitStack()
    P = ctx.enter_context(tc.tile_pool(name="persist", bufs=1))
    ST = ctx.enter_context(tc.tile_pool(name="stats", bufs=2))
    PS = ctx.enter_context(tc.tile_pool(name="psum", bufs=6, space="PSUM"))
    PSS = ctx.enter_context(tc.tile_pool(name="psmall", bufs=1, space="PSUM"))
    DR = ctx.enter_context(tc.tile_pool(name="dram", bufs=2, space="DRAM"))

    dma = nc.sync.dma_start

    bufA = P.tile([128, NBLK * WPAD], f16)
    bufB = P.tile([128, NBLK * WPAD], f16)
    bufC = P.tile([128, NBLK * WPAD], f16)
    raw = P.tile([128, NBLK * 384], f16)
    mask = P.tile([128, NBLK], f32)
    eyeS = P.tile([128, 64], f32)
    eyeD = P.tile([64, 128], f32)
    gpair = P.tile([64, 3], f32)
    bpair = P.tile([64, 3], f32)
    gres = P.tile([64, 10], f32)
    bres = P.tile([64, 10], f32)
    w3sb = P.tile([128, 2, 128], f16)
    eps = P.tile([128, 1], f32)

    dma(out=mask[:], in_=ten["mask_e"].ap())
    dma(out=eyeS[:], in_=ten["eyeS_e"].ap())
    dma(out=eyeD[:], in_=ten["eyeD_e"].ap())
    dma(out=gpair[:], in_=ten["gp_e"].ap())
    dma(out=bpair[:], in_=ten["bp_e"].ap())
    dma(out=gres[:], in_=ten["gr_e"].ap())
    dma(out=bres[:], in_=ten["br_e"].ap())
    dma(out=w3sb[:], in_=ten["w3_e"].ap().rearrange("t k m -> k t m"))
    nc.vector.memset(eps[:], float(EPS))

    nc.vector.memset(bufA[:], 0.0)
    nc.gpsimd.memset(bufB[:], 0.0)
    nc.vector.memset(bufC[:], 0.0)

    def all_reduce(ms, width):
        arin = DR.tile([128, width], f32, tag="arin")
        arout = DR.tile([128, width], f32, tag="arout", addr_space="Shared")
        dma(out=arin[:], in_=ms[:, 0:width])
        nc.gpsimd.collective_compute(
            "AllReduce", OP.add, replica_groups=RG,
            ins=[arin[:].opt()], outs=[arout[:].opt()])
        armv = ST.tile([128, width], f32, tag="armv")
        dma(out=armv[:, 0:width], in_=arout[:])
        return armv

    def stats_owned():
        """bn_stats over owned blocks of raw -> [128,2] sbuf [mean, msq]."""
        st6 = ST.tile([128, 18, 6], f32, tag="st6")
        for i in range(18):
            nc.vector.bn_stats(out=st6[:, i, :],
                               in_=raw[:, 4608 + 512 * i:4608 + 512 * (i + 1)])
        mv = ST.tile([128, 2], f32, tag="mv")
        nc.vector.bn_aggr(out=mv[:], in_=st6[:])
        ms = ST.tile([128, 2], f32, tag="ms")
        nc.vector.tensor_copy(out=ms[:, 0:1], in_=mv[:, 0:1])
        nc.vector.scalar_tensor_tensor(out=ms[:, 1:2], in0=mv[:, 0:1],
                                       scalar=mv[:, 0:1], in1=mv[:, 1:2],
                                       op0=OP.mult, op1=OP.add)
        return ms

    def stats_to_ab(armv, g64, b64):
        """armv [128,2] (sum over 8 cores of per-partition [mean,msq]) ->
        ab [128,2] f32: col0 A = g*rstd, col1 B' = gmean*A - beta."""
        pm = PSS.tile([64, 2], f32, tag="pm")
        nc.tensor.matmul(pm[:], eyeS[:, :], armv[:, 0:2], start=True, stop=True)
        pmv = ST.tile([64, 2], f32, tag="pmv")
        nc.scalar.activation(out=pmv[:], in_=pm[:], func=AF.Copy)
        ab64 = ST.tile([64, 2], f32, tag="ab64")
        varm = ST.tile([64, 1], f32, tag="varm")
        nc.vector.scalar_tensor_tensor(out=varm[:], in0=pmv[:, 0:1],
                                       scalar=pmv[:, 0:1], in1=pmv[:, 1:2],
                                       op0=OP.mult, op1=OP.subtract)  # -var
        nc.scalar.activation(out=varm[:], in_=varm[:], func=AF.Sqrt,
                             scale=-1.0, bias=eps[0:64, 0:1])
        nc.vector.reciprocal(out=varm[:], in_=varm[:])                # rstd
        nc.vector.tensor_tensor(out=ab64[:, 0:1], in0=g64, in1=varm[:],
                                op=OP.mult)                           # A
        nc.vector.scalar_tensor_tensor(out=ab64[:, 1:2], in0=pmv[:, 0:1],
                                       scalar=ab64[:, 0:1], in1=b64,
                                       op0=OP.mult, op1=OP.subtract)  # B'
        pd = PSS.tile([128, 2], f32, tag="pd")
        nc.tensor.matmul(pd[:], eyeD[:, :], ab64[:, :], start=True, stop=True)
        ab = ST.tile([128, 2], f32, tag="ab")
        nc.vector.tensor_copy(out=ab[:], in_=pd[:])
        return ab

    def norm_tail(ab, tgt, b0, b1, res=None):
        """z = A*raw - B'*mask (gpsimd), Lrelu in place (scalar), +res (vector)."""
        Bm = ST.tile([128, NBLK], f32, tag="Bm")
        nc.vector.tensor_scalar(out=Bm[:], in0=mask[:], scalar1=ab[:, 1:2],
                                scalar2=None, op0=OP.mult)
        tv = _padview(tgt, b0, b1)
        nc.vector.scalar_tensor_tensor(out=tv, in0=_rawview(raw, b0, b1),
                                       scalar=ab[:, 0:1], in1=_bcast(Bm, b0, b1),
                                       op0=OP.mult, op1=OP.subtract)
        nc.scalar.activation(out=tv, in_=tv, func=AF.Lrelu, alpha=0.01)
        if res is not None:
            nc.vector.tensor_tensor(out=tv, in0=tv, in1=_padview(res, b0, b1),
                                    op=OP.add)

    def evict(pp, b, n=392, lo=PADL):
        """psum block -> raw, masked per block (ACT: Copy with scale AP)."""
        nc.scalar.activation(out=raw[:, b * 384:(b + 1) * 384],
                             in_=pp[:, lo:lo + 384], func=AF.Copy,
                             scale=mask[:, b:b + 1])

    # ---------------- x2 stats pass (owned rows) + AR ----------------
    PAIRP = tc.alloc_tile_pool(name="pairp", bufs=2)
    st6a = PAIRP.tile([128, 36, 6], f32, bufs=1)
    st6b = PAIRP.tile([82, 36, 6], f32, bufs=1)
    for t in range(12):
        xa = PAIRP.tile([128, 4 * 384], f16, tag="xa", name=f"xas{t}")
        xb = PAIRP.tile([82, 4 * 384], f16, tag="xb", name=f"xbs{t}")
        dma(out=xa[:].rearrange("p (r l) -> p r l", l=L),
            in_=ten["x2_e"].ap()[0:128, 24 + 4 * t:28 + 4 * t, :])
        nc.gpsimd.dma_start(out=xb[:].rearrange("p (r l) -> p r l", l=L),
                            in_=ten["x2_e"].ap()[128:210, 24 + 4 * t:28 + 4 * t, :])
        for i in range(3):
            nc.vector.bn_stats(out=st6a[:, 3 * t + i, :],
                               in_=xa[:, 512 * i:512 * (i + 1)])
            nc.vector.bn_stats(out=st6b[:, 3 * t + i, :],
                               in_=xb[:, 512 * i:512 * (i + 1)])
    mvx = ST.tile([128, 4], f32, tag="mvx")
    nc.vector.memset(mvx[:], 0.0)
    mva = ST.tile([128, 2], f32, tag="mv")
    nc.vector.bn_aggr(out=mva[:], in_=st6a[:])
    mvb = ST.tile([82, 2], f32, tag="mvb")
    nc.vector.bn_aggr(out=mvb[:], in_=st6b[:])
    nc.vector.tensor_copy(out=mvx[:, 0:1], in_=mva[:, 0:1])
    nc.vector.scalar_tensor_tensor(out=mvx[:, 1:2], in0=mva[:, 0:1],
                                   scalar=mva[:, 0:1], in1=mva[:, 1:2],
                                   op0=OP.mult, op1=OP.add)
    nc.vector.tensor_copy(out=mvx[0:82, 2:3], in_=mvb[:, 0:1])
    nc.vector.scalar_tensor_tensor(out=mvx[0:82, 3:4], in0=mvb[:, 0:1],
                                   scalar=mvb[:, 0:1], in1=mvb[:, 1:2],
                                   op0=OP.mult, op1=OP.add)
    armx = all_reduce(mvx, 4)   # AR#1 (in flight during pair1)

    # ---------------- x1 norm + pair1 into bufA ----------------
    x1sb = PAIRP.tile([128, 7, L], f16, bufs=1)
    x1n = PAIRP.tile([128, 7, L], f16, bufs=1)
    x1ls = PAIRP.tile([128, 7, 96], f16, bufs=1)
    x1nl = PAIRP.tile([128, 7, 96], f16, bufs=1)
    w1a = PAIRP.tile([128, 7, 64], f16, bufs=1)
    w1c = PAIRP.tile([128, 7, 128], f16, bufs=1)
    dma(out=x1sb[:], in_=ten["x1_e"].ap().rearrange("(a p) l -> p a l", p=128))
    dma(out=x1ls[:], in_=ten["x1l_e"].ap().rearrange("(a p) l -> p a l", p=128))
    dma(out=w1a[:], in_=ten["w1a_e"].ap().rearrange("a k m -> k a m"))
    dma(out=w1c[:], in_=ten["w1c_e"].ap().rearrange("a k m -> k a m"))
    st1 = ST.tile([128, 6], f32, tag="st1")
    mv1 = ST.tile([128, 7, 2], f32, tag="mv1")
    for j in range(7):
        nc.vector.bn_stats(out=st1[:], in_=x1sb[:, j, :])
        nc.vector.bn_aggr(out=mv1[:, j, :], in_=st1[:])
    rst1 = ST.tile([128, 7], f32, tag="rst1")
    nc.scalar.activation(out=rst1[:], in_=mv1[:, :, 1], func=AF.Sqrt,
                         bias=eps[:, 0:1])
    nc.vector.reciprocal(out=rst1[:], in_=rst1[:])
    for j in range(7):
        nc.vector.tensor_scalar(out=x1n[:, j, :], in0=x1sb[:, j, :],
                                scalar1=mv1[:, j, 0:1], scalar2=rst1[:, j:j + 1],
                                op0=OP.subtract, op1=OP.mult)
        nc.vector.tensor_scalar(out=x1nl[:, j, :], in0=x1ls[:, j, :],
                                scalar1=mv1[:, j, 0:1], scalar2=rst1[:, j:j + 1],
                                op0=OP.subtract, op1=OP.mult)

    rowg_p = PS.tile([64, L], f32, tag="pp")
    rowl_p = PS.tile([64, 96], f32, tag="pp")
    colD_p = PS.tile([128, L], f32, tag="pp")
    for j in range(7):
        kw = dict(start=(j == 0), stop=(j == 6))
        nc.tensor.matmul(rowg_p[:], w1a[:, j, :], x1n[:, j, :], **kw)
        nc.tensor.matmul(rowl_p[:], w1a[:, j, :], x1nl[:, j, :], **kw)
        nc.tensor.matmul(colD_p[:], w1c[:, j, :], x1n[:, j, :], **kw)
    colD = PAIRP.tile([128, L], f16, bufs=1)
    nc.scalar.activation(out=colD[:], in_=colD_p[:], func=AF.Copy)
    rl32 = PAIRP.tile([64, 96], f32, bufs=1)
    nc.scalar.activation(out=rl32[:], in_=rowl_p[:], func=AF.Copy)
    rowv = PAIRP.tile([128, NBLK], f32, bufs=1)
    dma(out=rowv[0:64, :], in_=rl32[:, 0::2])
    dma(out=rowv[64:128, :], in_=rl32[:, 1::2])

    # pair1 stats (exact, local): mean/var = row stats + col stats
    strc = ST.tile([64, 6], f32, tag="strc")
    mvp1 = ST.tile([64, 2], f32, tag="mvp1")
    mvc1 = ST.tile([64, 2], f32, tag="mvc1")
    nc.vector.bn_stats(out=strc[:], in_=rowg_p[:, :])
    nc.vector.bn_aggr(out=mvp1[:], in_=strc[:])
    nc.vector.bn_stats(out=strc[:], in_=colD[0:64, :])
    nc.vector.bn_aggr(out=mvc1[:], in_=strc[:])
    nc.vector.tensor_tensor(out=mvp1[:], in0=mvp1[:], in1=mvc1[:], op=OP.add)
    # A1 = g1*rstd, B1' = mean*A1 - b1  (no AR, no phase combine needed)
    ab64 = ST.tile([64, 2], f32, tag="ab64")
    varm = ST.tile([64, 1], f32, tag="varm")
    nc.scalar.activation(out=varm[:], in_=mvp1[:, 1:2], func=AF.Sqrt,
                         bias=eps[0:64, 0:1])
    nc.vector.reciprocal(out=varm[:], in_=varm[:])
    nc.vector.tensor_tensor(out=ab64[:, 0:1], in0=gpair[:, 0:1], in1=varm[:],
                            op=OP.mult)
    nc.vector.scalar_tensor_tensor(out=ab64[:, 1:2], in0=mvp1[:, 0:1],
                                   scalar=ab64[:, 0:1], in1=bpair[:, 0:1],
                                   op0=OP.mult, op1=OP.subtract)
    pd1 = PSS.tile([128, 2], f32, tag="pd")
    nc.tensor.matmul(pd1[:], eyeD[:, :], ab64[:, :], start=True, stop=True)
    ab1 = ST.tile([128, 2], f32, tag="ab")
    nc.vector.tensor_copy(out=ab1[:], in_=pd1[:])
    # Rv = A1*rowv - B1' ; p1 = lrelu(colD*A1 + Rv)
    Rv = ST.tile([128, NBLK], f32, tag="Bm")
    nc.vector.scalar_tensor_tensor(out=Rv[:], in0=rowv[:], scalar=ab1[:, 0:1],
                                   in1=_bcast_b1(ab1), op0=OP.mult,
                                   op1=OP.subtract)
    p1v = _padview(bufA, PB0, PB1)
    colD_bc = bass.AP(tensor=colD[:, :].tensor, offset=colD[:, :].offset,
                      ap=[colD[:, :].ap[0], [0, PB1 - PB0], [1, 384]])
    nc.vector.scalar_tensor_tensor(out=p1v, in0=colD_bc, scalar=ab1[:, 0:1],
                                   in1=_bcast(Rv, PB0, PB1),
                                   op0=OP.mult, op1=OP.add)
    nc.scalar.activation(out=p1v, in_=p1v, func=AF.Lrelu, alpha=0.01)

    # ---------------- pair2: scale W2 by rstd(x2), matmul pass ----------------
    w2a = PAIRP.tile([128, 64], f16, bufs=1)
    w2b = PAIRP.tile([128, 64], f16, bufs=1)
    dma(out=w2a[:], in_=ten["w2a_e"].ap())
    dma(out=w2b[:], in_=ten["w2b_e"].ap())
    # rstd for both groups from armx [128,4] = sum over cores [mA,qA,mB,qB]
    mg = ST.tile([128, 2], f32, tag="mg")
    vg = ST.tile([128, 2], f32, tag="vg")
    nc.vector.tensor_scalar(out=mg[:], in0=armx[:, 0::2], scalar1=0.125,
                            scalar2=None, op0=OP.mult)
    nc.vector.tensor_scalar(out=vg[:], in0=armx[:, 1::2], scalar1=0.125,
                            scalar2=None, op0=OP.mult)
    nc.vector.scalar_tensor_tensor(out=vg[:, 0:1], in0=mg[:, 0:1],
                                   scalar=mg[:, 0:1], in1=vg[:, 0:1],
                                   op0=OP.mult, op1=OP.subtract)  # m^2-q = -var
    nc.vector.scalar_tensor_tensor(out=vg[:, 1:2], in0=mg[:, 1:2],
                                   scalar=mg[:, 1:2], in1=vg[:, 1:2],
                                   op0=OP.mult, op1=OP.subtract)
    nc.scalar.activation(out=vg[:], in_=vg[:], func=AF.Sqrt, scale=-1.0,
                         bias=eps[:, 0:1])
    nc.vector.reciprocal(out=vg[:], in_=vg[:])   # rstd [128,2]
    w2as = PAIRP.tile([128, 64], f16, bufs=1)
    w2bs = PAIRP.tile([128, 64], f16, bufs=1)
    nc.vector.tensor_scalar(out=w2as[:], in0=w2a[:], scalar1=vg[:, 0:1],
                            scalar2=None, op0=OP.mult)
    nc.vector.tensor_scalar(out=w2bs[:], in0=w2b[:], scalar1=vg[:, 1:2],
                            scalar2=None, op0=OP.mult)

    for ci in range(22):
        xa = PAIRP.tile([128, 4 * 384], f16, tag="xa", name=f"xam{ci}")
        xb = PAIRP.tile([82, 4 * 384], f16, tag="xb", name=f"xbm{ci}")
        dma(out=xa[:].rearrange("p (r l) -> p r l", l=L),
            in_=ten["x2_e"].ap()[0:128, 4 + 4 * ci:8 + 4 * ci, :])
        nc.gpsimd.dma_start(out=xb[:].rearrange("p (r l) -> p r l", l=L),
                            in_=ten["x2_e"].ap()[128:210, 4 + 4 * ci:8 + 4 * ci, :])
        for t in range(2):
            b = PB0 + 2 * ci + t
            pp = PS.tile([128, 392], f32, tag="pp")
            er, od = 2 * t, 2 * t + 1
            nc.tensor.matmul(pp[0:64, 0:384], w2as[:, :],
                             xa[:, er * 384:(er + 1) * 384], start=True, stop=False)
            nc.tensor.matmul(pp[0:64, 0:384], w2bs[0:82, :],
                             xb[0:82, er * 384:(er + 1) * 384], start=False, stop=True)
            nc.tensor.matmul(pp[64:128, 0:384], w2as[:, :],
                             xa[:, od * 384:(od + 1) * 384], start=True, stop=False,
                             tile_position=(0, 64))
            nc.tensor.matmul(pp[64:128, 0:384], w2bs[0:82, :],
                             xb[0:82, od * 384:(od + 1) * 384], start=False, stop=True,
                             tile_position=(0, 64))
            evict(pp, b, lo=0)
    ms2 = stats_owned()
    arm2 = all_reduce(ms2, 2)
    ab2 = stats_to_ab(arm2, gpair[:, 1:2], bpair[:, 1:2])
    norm_tail(ab2, bufB, PB0, PB1)
    PAIRP.release()
    WP = tc.alloc_tile_pool(name="respool", bufs=2)

    # ---------------- pair3 into bufC ----------------
    for g0 in range(PB0, PB1, 6):
        g1 = min(g0 + 6, PB1)
        pps = []
        for b in range(g0, g1):
            pp = PS.tile([128, 392], f32, tag="pp")
            pps.append(pp)
            nc.tensor.matmul(pp[:], w3sb[:, 0, :],
                             bufA[:, b * WPAD:(b + 1) * WPAD], start=True, stop=False)
            nc.tensor.matmul(pp[:], w3sb[:, 1, :],
                             bufB[:, b * WPAD:(b + 1) * WPAD], start=False, stop=True)
        for i, b in enumerate(range(g0, g1)):
            evict(pps[i], b)
    ms3 = stats_owned()
    arm3 = all_reduce(ms3, 2)
    ab3 = stats_to_ab(arm3, gpair[:, 2:3], bpair[:, 2:3])
    norm_tail(ab3, bufC, PB0, PB1)

    # ---------------- ResNet: 5 blocks x 2 convs ----------------
    cur, tmp, nxt = bufC, bufA, bufB
    for q in range(10):
        d = DILS[q]
        b0, b1 = B0[q], B1[q]
        IN = cur if q % 2 == 0 else tmp
        TGT = tmp if q % 2 == 0 else nxt
        wf = WP.tile([128, 9, 128], f16, tag="wf")
        dma(out=wf[:], in_=ten["wf_e"].ap()[q].rearrange("t k m -> k t m"))
        if d == 1:
            wh = WP.tile([128, 6, 64], f16, tag="wh")
            dma(out=wh[:], in_=ten["wh_e"].ap()[q].rearrange("t k m -> k t m"))

        # taps: list of (kind, lhsT-getter, rhs partition range, delta, tilepos, outslice)
        taps = []
        if d != 1:
            for ki in range(3):
                for kj in range(3):
                    dd = (ki - 1) * (d // 2) * WPAD + (kj - 1) * d
                    taps.append(("f", ki * 3 + kj, dd))
        else:
            for kj in range(3):
                taps.append(("f", 3 + kj, kj - 1))
            for ki in (0, 2):
                for kj in range(3):
                    t = (0 if ki == 0 else 3) + kj
                    d0 = (-WPAD if ki == 0 else 0) + (kj - 1)
                    d1_ = (0 if ki == 0 else WPAD) + (kj - 1)
                    taps.append(("h", t, (d0, d1_)))
        ntap = len(taps)

        for g0 in range(b0, b1, 6):
            g1 = min(g0 + 6, b1)
            pps = {b: PS.tile([128, 392], f32, tag="pp", name=f"pp_{q}_{b}")
                   for b in range(g0, g1)}
            for it, (kind, t, dd) in enumerate(taps):
                first, last = it == 0, it == ntap - 1
                for b in range(g0, g1):
                    pp = pps[b]
                    base = b * WPAD
                    if kind == "f":
                        nc.tensor.matmul(pp[:], wf[:, t, :],
                                         IN[:, base + dd:base + dd + WPAD],
                                         start=first, stop=False,
                                         skip_group_check=True)
                    else:
                        d0, d1_ = dd
                        nc.tensor.matmul(pp[0:64, :], wh[64:128, t, :],
                                         IN[64:128, base + d0:base + d0 + WPAD],
                                         start=False, stop=False,
                                         tile_position=(64, 0),
                                         skip_group_check=True)
                        nc.tensor.matmul(pp[64:128, :], wh[0:64, t, :],
                                         IN[0:64, base + d1_:base + d1_ + WPAD],
                                         start=False, stop=last,
                                         tile_position=(0, 64),
                                         skip_group_check=True)
            for b in range(g0, g1):
                evict(pps[b], b)
        ms = stats_owned()
        arm = all_reduce(ms, 2)
        ab = stats_to_ab(arm, gres[:, q:q + 1], bres[:, q:q + 1])
        norm_tail(ab, TGT, b0, b1, res=(cur if q % 2 == 1 else None))
        if q % 2 == 1:
            cur, nxt = nxt, cur

    # ---------------- output: owned blocks, fp32, de-phase ----------------
    for hf in range(2):
        ob0 = OB0 + 12 * hf
        o32 = WP.tile([128, 12 * 384], f32, tag="o32", bufs=1, name=f"o32_{hf}")
        nc.vector.tensor_copy(out=o32[:].rearrange("p (b l) -> p b l", l=384),
                              in_=_padview(cur, ob0, ob0 + 12))
        oap = ten["out_e"].ap()
        dma(out=oap[:, 24 * hf + 0:24 * hf + 24:2, :],
            in_=o32[0:64, :].rearrange("p (b l) -> p b l", l=384))
        dma(out=oap[:, 24 * hf + 1:24 * hf + 24:2, :],
            in_=o32[64:128, :].rearrange("p (b l) -> p b l", l=384))
    WP.release()
    ctx.close()


def _bcast_b1(ab1):
    base = ab1[:, 1:2]
    return bass.AP(tensor=base.tensor, offset=base.offset,
                   ap=[base.ap[0], [0, NBLK]])


# ======================= host side =======================

_NC = None


def _get_nc():
    global _NC
    if _NC is None:
        _NC = build()
    return _NC


def _prep(x_1d, x_2d, W1, g1, b1, W2, g2, b2, W3, g3, b3,
          res_w, res_b, res_g, res_beta):
    n16 = lambda a: np.ascontiguousarray(np.asarray(a, np.float32).astype(np.float16))
    n32 = lambda a: np.ascontiguousarray(np.asarray(a, np.float32))

    x1 = np.zeros((896, L), np.float16)
    x1[:D1] = n16(x_1d[0])
    x2f = n16(x_2d[0])                      # [210, 384, 384]

    W1 = np.asarray(W1, np.float32)
    w1a = np.zeros((7, 128, 64), np.float16)
    w1c = np.zeros((7, 128, 128), np.float16)
    for a in range(7):
        r0, r1 = a * 128, min((a + 1) * 128, D1)
        w1a[a, :r1 - r0, :] = W1[:, :D1][:, r0:r1].T.astype(np.float16)
        wb = W1[:, D1:][:, r0:r1].T.astype(np.float16)
        w1c[a, :r1 - r0, 0:64] = wb
        w1c[a, :r1 - r0, 64:128] = wb
    W2 = np.asarray(W2, np.float32)
    w2a = np.ascontiguousarray(W2[:, 0:128].T.astype(np.float16))
    w2b = np.zeros((128, 64), np.float16)
    w2b[0:82] = W2[:, 128:210].T.astype(np.float16)
    W3 = np.asarray(W3, np.float32)
    e2 = np.eye(2, dtype=np.float32)
    w3 = np.stack([np.kron(e2, W3[:, :CH].T), np.kron(e2, W3[:, CH:].T)])
    w3 = w3.astype(np.float16)
    wf = np.zeros((10, 9, 128, 128), np.float16)
    wh = np.zeros((10, 6, 128, 64), np.float16)
    res_w = np.asarray(res_w, np.float32)
    for q in range(10):
        w = res_w[q // 2, q % 2]            # [O, I, 3, 3]
        dq = DILS[q]
        for ki in range(3):
            for kj in range(3):
                tT = w[:, :, ki, kj].T.astype(np.float16)   # [I, O]
                if dq != 1 or ki == 1:
                    wf[q, ki * 3 + kj] = np.kron(e2, tT)
                else:
                    t = (0 if ki == 0 else 3) + kj
                    wh[q, t, 0:64] = tT
                    wh[q, t, 64:128] = tT
    eyeS = np.zeros((128, 64), np.float32)
    eyeD = np.zeros((64, 128), np.float32)
    for m in range(64):
        eyeS[m, m] = eyeS[m + 64, m] = 1.0 / 16.0
        eyeD[m, m] = eyeD[m, m + 64] = 1.0
    gpair = n32(np.stack([g1, g2, g3], 1))
    bpair = n32(np.stack([b1, b2, b3], 1))
    gresv = n32(np.asarray(res_g, np.float32).reshape(10, CH).T)
    bresv = n32(np.asarray(res_beta, np.float32).reshape(10, CH).T)

    common = dict(x1=x1, w1a=w1a, w1c=w1c, w2a=w2a, w2b=w2b, w3=w3, wf=wf,
                  wh=wh, eyeS=eyeS, eyeD=eyeD, gpair=gpair, bpair=bpair,
                  gres=gresv, bres=bresv)
    in_maps = []
    for c in range(NCOR):
        r0 = 48 * c - 24
        x2s = np.zeros((D2, 96, L), np.float16)
        lo, hi = max(0, r0), min(L, r0 + 96)
        x2s[:, lo - r0:hi - r0, :] = x2f[:, lo:hi, :]
        x1l = np.zeros((896, 96), np.float16)
        x1l[:, lo - r0:hi - r0] = x1[:, lo:hi]
        msk = np.zeros((128, NBLK), np.float32)
        for p in range(2):
            for b in range(NBLK):
                r = r0 + 2 * b + p
                if 0 <= r < L:
                    msk[64 * p:64 * (p + 1), b] = 1.0
        in_maps.append(dict(common, x2=x2s, x1loc=x1l, mask=msk))
    return in_maps


def _run(inputs, trace=False):
    nc = _get_nc()
    in_maps = _prep(**inputs)
    res = run_bass_kernel_spmd(nc, in_maps, core_ids=list(range(NCOR)),
                               trace=trace)
    out = np.empty((1, CH, L, L), np.float32)
    for c in range(NCOR):
        out[0, :, 48 * c:48 * (c + 1), :] = res.results[c]["out"]
    return out, res


def kernel(**inputs):
    out, _ = _run(inputs, trace=False)
    return out


# revision 8
# speedup vs baseline: 1.1443x; 1.1443x over previous
"""nn_Intra_ResNet on 8 TRN2 NeuronCores (Bass/Tile, SPMD).

Row-sharded 8-way (48 rows/core) with 20-row halo recompute (no halo
exchange). Activations live in SBUF phase-packed: partition = ch + 64*(row
parity), column = block*392 + 4 + j  (a "block" is a pair of image rows,
392 = 384 + 2*4 zero pad columns). Every 3x3 dilated conv tap is then a
K=128/M=128 fp16 matmul at a column offset (even-d taps and d=1 center
row), or a pair of concurrent 64x64 quadrant matmuls via tile_position
(d=1 phase-flip taps). InstanceNorm: per-partition bn_stats + one [128,2]
fp32 AllReduce per norm (13 total), then a fused scale/bias (+mask) +
LeakyReLU applied with 3 big strided ops (gpsimd z-pass, scalar Lrelu,
vector residual-add).
"""
import sys

for _p in ("/opt/trn_rl_repo",):
    if _p not in sys.path:
        sys.path.insert(0, _p)

import numpy as np

import concourse.bass as bass
import concourse.tile as tile
from concourse import bacc, mybir
from concourse.bass_utils import run_bass_kernel_spmd

f16 = mybir.dt.float16
f32 = mybir.dt.float32
AF = mybir.ActivationFunctionType
OP = mybir.AluOpType

NCOR = 8
L = 384
CH = 64
D1 = 788
D2 = 210
EPS = 1e-5
DILS = [1, 1, 2, 2, 4, 4, 2, 2, 1, 1]
OWN = 48          # rows per core
NBLK = 48         # buffer blocks (96 rows incl 2 guard blocks/side)
WPAD = 392        # padded row width (4 | 384 | 4)
PADL = 4
RG = [list(range(NCOR))]

# halo rows needed before conv q; conv q computes local rows [24-H[q+1], 72+H[q+1])
H = [sum(DILS[i:]) for i in range(11)]          # [20,19,18,16,14,10,6,4,2,1,0]
B0 = [(24 - H[q + 1]) // 2 for q in range(10)]  # [2,3,4,5,7,9,10,11,11,12]
B1 = [(72 + H[q + 1] + 1) // 2 for q in range(10)]
PB0, PB1 = 2, 46   # pair-stage computed blocks
OB0, OB1 = 12, 36  # owned blocks


def _padview(buf, b0, b1):
    """[128, b1-b0, 384] view of padded-layout buffer (data cols only)."""
    base = buf[:, 0:1]
    return bass.AP(tensor=base.tensor, offset=base.offset + b0 * WPAD + PADL,
                   ap=[base.ap[0], [WPAD, b1 - b0], [1, 384]])


def _bcast(t2d, b0, b1):
    """[128, b1-b0, 384] broadcast view of a [128, NBLK] per-block tile."""
    base = t2d[:, b0:b1]
    return bass.AP(tensor=base.tensor, offset=base.offset,
                   ap=[base.ap[0], base.ap[1], [0, 384]])


def _rawview(raw, b0, b1):
    return raw[:, b0 * 384:b1 * 384].rearrange("p (b l) -> p b l", l=384)


def build():
    nc = bacc.Bacc("TRN2", target_bir_lowering=False, debug=False,
                   num_devices=NCOR)
    ein = dict(kind="ExternalInput")
    x1_e = nc.dram_tensor("x1", [896, L], f16, **ein)
    x1l_e = nc.dram_tensor("x1loc", [896, 96], f16, **ein)
    x2_e = nc.dram_tensor("x2", [D2, 96, L], f16, **ein)
    mask_e = nc.dram_tensor("mask", [128, NBLK], f32, **ein)
    w1a_e = nc.dram_tensor("w1a", [7, 128, 64], f16, **ein)
    w1c_e = nc.dram_tensor("w1c", [7, 128, 128], f16, **ein)
    w2a_e = nc.dram_tensor("w2a", [128, 64], f16, **ein)
    w2b_e = nc.dram_tensor("w2b", [128, 64], f16, **ein)
    w3_e = nc.dram_tensor("w3", [2, 128, 128], f16, **ein)
    wf_e = nc.dram_tensor("wf", [10, 9, 128, 128], f16, **ein)
    wh_e = nc.dram_tensor("wh", [10, 6, 128, 64], f16, **ein)
    eyeS_e = nc.dram_tensor("eyeS", [128, 64], f32, **ein)
    eyeD_e = nc.dram_tensor("eyeD", [64, 128], f32, **ein)
    gp_e = nc.dram_tensor("gpair", [64, 3], f32, **ein)
    bp_e = nc.dram_tensor("bpair", [64, 3], f32, **ein)
    gr_e = nc.dram_tensor("gres", [64, 10], f32, **ein)
    br_e = nc.dram_tensor("bres", [64, 10], f32, **ein)
    out_e = nc.dram_tensor("out", [CH, OWN, L], f32, kind="ExternalOutput")

    with tile.TileContext(nc) as tc:
        _body(nc, tc, locals())
    nc.compile()
    return nc


def _body(nc, tc, ten):
    from contextlib import ExitStack
    ctx = ExitStack()
    P = ctx.enter_context(tc.tile_pool(name="persist", bufs=1))
    ST = ctx.enter_context(tc.tile_pool(name="stats", bufs=2))
    PS = ctx.enter_context(tc.tile_pool(name="psum", bufs=6, space="PSUM"))
    PSS = ctx.enter_context(tc.tile_pool(name="psmall", bufs=1, space="PSUM"))
    DR = ctx.enter_context(tc.tile_pool(name="dram", bufs=2, space="DRAM"))

    dma = nc.sync.dma_start

    bufA = P.tile([128, NBLK * WPAD], f16)
    bufB = P.tile([128, NBLK * WPAD], f16)
    bufC = P.tile([128, NBLK * WPAD], f16)
    raw = P.tile([128, NBLK * 384], f16)
    mask = P.tile([128, NBLK], f32)
    eyeS = P.tile([128, 64], f32)
    eyeD = P.tile([64, 128], f32)
    gpair = P.tile([64, 3], f32)
    bpair = P.tile([64, 3], f32)
    gres = P.tile([64, 10], f32)
    bres = P.tile([64, 10], f32)
    w3sb = P.tile([128, 2, 128], f16)
    eps = P.tile([128, 1], f32)

    dma(out=mask[:], in_=ten["mask_e"].ap())
    dma(out=eyeS[:], in_=ten["eyeS_e"].ap())
    dma(out=eyeD[:], in_=ten["eyeD_e"].ap())
    dma(out=gpair[:], in_=ten["gp_e"].ap())
    dma(out=bpair[:], in_=ten["bp_e"].ap())
    dma(out=gres[:], in_=ten["gr_e"].ap())
    dma(out=bres[:], in_=ten["br_e"].ap())
    dma(out=w3sb[:], in_=ten["w3_e"].ap().rearrange("t k m -> k t m"))
    nc.vector.memset(eps[:], float(EPS))

    nc.vector.memset(bufA[:], 0.0)
    nc.gpsimd.memset(bufB[:], 0.0)
    nc.vector.memset(bufC[:], 0.0)

    def all_reduce(ms, width):
        arin = DR.tile([128, width], f32, tag="arin")
        arout = DR.tile([128, width], f32, tag="arout", addr_space="Shared")
        dma(out=arin[:], in_=ms[:, 0:width])
        nc.gpsimd.collective_compute(
            "AllReduce", OP.add, replica_groups=RG,
            ins=[arin[:].opt()], outs=[arout[:].opt()])
        armv = ST.tile([128, width], f32, tag="armv")
        dma(out=armv[:, 0:width], in_=arout[:])
        return armv

    def stats_owned():
        """bn_stats over owned blocks of raw -> [128,2] sbuf [mean, msq]."""
        st6 = ST.tile([128, 18, 6], f32, tag="st6")
        for i in range(18):
            nc.vector.bn_stats(out=st6[:, i, :],
                               in_=raw[:, 4608 + 512 * i:4608 + 512 * (i + 1)])
        mv = ST.tile([128, 2], f32, tag="mv")
        nc.vector.bn_aggr(out=mv[:], in_=st6[:])
        ms = ST.tile([128, 2], f32, tag="ms")
        nc.vector.tensor_copy(out=ms[:, 0:1], in_=mv[:, 0:1])
        nc.vector.scalar_tensor_tensor(out=ms[:, 1:2], in0=mv[:, 0:1],
                                       scalar=mv[:, 0:1], in1=mv[:, 1:2],
                                       op0=OP.mult, op1=OP.add)
        return ms

    def stats_to_ab(armv, g64, b64):
        """armv [128,2] (sum over 8 cores of per-partition [mean,msq]) ->
        ab [128,2] f32: col0 A = g*rstd, col1 B' = gmean*A - beta."""
        pm = PSS.tile([64, 2], f32, tag="pm")
        nc.tensor.matmul(pm[:], eyeS[:, :], armv[:, 0:2], start=True, stop=True)
        pmv = ST.tile([64, 2], f32, tag="pmv")
        nc.scalar.activation(out=pmv[:], in_=pm[:], func=AF.Copy)
        ab64 = ST.tile([64, 2], f32, tag="ab64")
        varm = ST.tile([64, 1], f32, tag="varm")
        nc.vector.scalar_tensor_tensor(out=varm[:], in0=pmv[:, 0:1],
                                       scalar=pmv[:, 0:1], in1=pmv[:, 1:2],
                                       op0=OP.mult, op1=OP.subtract)  # -var
        nc.scalar.activation(out=varm[:], in_=varm[:], func=AF.Sqrt,
                             scale=-1.0, bias=eps[0:64, 0:1])
        nc.vector.reciprocal(out=varm[:], in_=varm[:])                # rstd
        nc.vector.tensor_tensor(out=ab64[:, 0:1], in0=g64, in1=varm[:],
                                op=OP.mult)                           # A
        nc.vector.scalar_tensor_tensor(out=ab64[:, 1:2], in0=pmv[:, 0:1],
                                       scalar=ab64[:, 0:1], in1=b64,
                                       op0=OP.mult, op1=OP.subtract)  # B'
        pd = PSS.tile([128, 2], f32, tag="pd")
        nc.tensor.matmul(pd[:], eyeD[:, :], ab64[:, :], start=True, stop=True)
        ab = ST.tile([128, 2], f32, tag="ab")
        nc.vector.tensor_copy(out=ab[:], in_=pd[:])
        return ab

    def norm_tail(ab, tgt, b0, b1, res=None, nch=4):
        """Chunked: z = A*raw - B'*mask (vector), Lrelu in place (scalar),
        +res (vector). Chunking lets the next conv's matmuls start after
        the first chunk instead of after the whole pass."""
        Bm = ST.tile([128, NBLK], f32, tag="Bm")
        nc.vector.tensor_scalar(out=Bm[:], in0=mask[:], scalar1=ab[:, 1:2],
                                scalar2=None, op0=OP.mult)
        nb = b1 - b0
        cuts = [b0 + (nb * i) // nch for i in range(nch)] + [b1]
        for c0, c1 in zip(cuts[:-1], cuts[1:]):
            if c1 <= c0:
                continue
            tv = _padview(tgt, c0, c1)
            nc.vector.scalar_tensor_tensor(out=tv, in0=_rawview(raw, c0, c1),
                                           scalar=ab[:, 0:1],
                                           in1=_bcast(Bm, c0, c1),
                                           op0=OP.mult, op1=OP.subtract)
            nc.scalar.activation(out=tv, in_=tv, func=AF.Lrelu, alpha=0.01)
            if res is not None:
                nc.vector.tensor_tensor(out=tv, in0=tv,
                                        in1=_padview(res, c0, c1), op=OP.add)

    def evict(pp, b, n=392, lo=PADL):
        """psum block -> raw, masked per block (ACT: Copy with scale AP)."""
        nc.scalar.activation(out=raw[:, b * 384:(b + 1) * 384],
                             in_=pp[:, lo:lo + 384], func=AF.Copy,
                             scale=mask[:, b:b + 1])

    # ---------------- x2 stats pass (owned rows) + AR ----------------
    PAIRP = tc.alloc_tile_pool(name="pairp", bufs=2)
    st6a = PAIRP.tile([128, 36, 6], f32, bufs=1)
    st6b = PAIRP.tile([82, 36, 6], f32, bufs=1)
    for t in range(12):
        xa = PAIRP.tile([128, 4 * 384], f16, tag="xa", name=f"xas{t}")
        xb = PAIRP.tile([82, 4 * 384], f16, tag="xb", name=f"xbs{t}")
        dma(out=xa[:].rearrange("p (r l) -> p r l", l=L),
            in_=ten["x2_e"].ap()[0:128, 24 + 4 * t:28 + 4 * t, :])
        nc.gpsimd.dma_start(out=xb[:].rearrange("p (r l) -> p r l", l=L),
                            in_=ten["x2_e"].ap()[128:210, 24 + 4 * t:28 + 4 * t, :])
        for i in range(3):
            nc.vector.bn_stats(out=st6a[:, 3 * t + i, :],
                               in_=xa[:, 512 * i:512 * (i + 1)])
            nc.vector.bn_stats(out=st6b[:, 3 * t + i, :],
                               in_=xb[:, 512 * i:512 * (i + 1)])
    mvx = ST.tile([128, 4], f32, tag="mvx")
    nc.vector.memset(mvx[:], 0.0)
    mva = ST.tile([128, 2], f32, tag="mv")
    nc.vector.bn_aggr(out=mva[:], in_=st6a[:])
    mvb = ST.tile([82, 2], f32, tag="mvb")
    nc.vector.bn_aggr(out=mvb[:], in_=st6b[:])
    nc.vector.tensor_copy(out=mvx[:, 0:1], in_=mva[:, 0:1])
    nc.vector.scalar_tensor_tensor(out=mvx[:, 1:2], in0=mva[:, 0:1],
                                   scalar=mva[:, 0:1], in1=mva[:, 1:2],
                                   op0=OP.mult, op1=OP.add)
    nc.vector.tensor_copy(out=mvx[0:82, 2:3], in_=mvb[:, 0:1])
    nc.vector.scalar_tensor_tensor(out=mvx[0:82, 3:4], in0=mvb[:, 0:1],
                                   scalar=mvb[:, 0:1], in1=mvb[:, 1:2],
                                   op0=OP.mult, op1=OP.add)
    armx = all_reduce(mvx, 4)   # AR#1 (in flight during pair1)

    # ---------------- x1 norm + pair1 into bufA ----------------
    x1sb = PAIRP.tile([128, 7, L], f16, bufs=1)
    x1n = PAIRP.tile([128, 7, L], f16, bufs=1)
    x1ls = PAIRP.tile([128, 7, 96], f16, bufs=1)
    x1nl = PAIRP.tile([128, 7, 96], f16, bufs=1)
    w1a = PAIRP.tile([128, 7, 64], f16, bufs=1)
    w1c = PAIRP.tile([128, 7, 128], f16, bufs=1)
    dma(out=x1sb[:], in_=ten["x1_e"].ap().rearrange("(a p) l -> p a l", p=128))
    dma(out=x1ls[:], in_=ten["x1l_e"].ap().rearrange("(a p) l -> p a l", p=128))
    dma(out=w1a[:], in_=ten["w1a_e"].ap().rearrange("a k m -> k a m"))
    dma(out=w1c[:], in_=ten["w1c_e"].ap().rearrange("a k m -> k a m"))
    st1 = ST.tile([128, 6], f32, tag="st1")
    mv1 = ST.tile([128, 7, 2], f32, tag="mv1")
    for j in range(7):
        nc.vector.bn_stats(out=st1[:], in_=x1sb[:, j, :])
        nc.vector.bn_aggr(out=mv1[:, j, :], in_=st1[:])
    rst1 = ST.tile([128, 7], f32, tag="rst1")
    nc.scalar.activation(out=rst1[:], in_=mv1[:, :, 1], func=AF.Sqrt,
                         bias=eps[:, 0:1])
    nc.vector.reciprocal(out=rst1[:], in_=rst1[:])
    for j in range(7):
        nc.vector.tensor_scalar(out=x1n[:, j, :], in0=x1sb[:, j, :],
                                scalar1=mv1[:, j, 0:1], scalar2=rst1[:, j:j + 1],
                                op0=OP.subtract, op1=OP.mult)
        nc.vector.tensor_scalar(out=x1nl[:, j, :], in0=x1ls[:, j, :],
                                scalar1=mv1[:, j, 0:1], scalar2=rst1[:, j:j + 1],
                                op0=OP.subtract, op1=OP.mult)

    rowg_p = PS.tile([64, L], f32, tag="pp")
    rowl_p = PS.tile([64, 96], f32, tag="pp")
    colD_p = PS.tile([128, L], f32, tag="pp")
    for j in range(7):
        kw = dict(start=(j == 0), stop=(j == 6))
        nc.tensor.matmul(rowg_p[:], w1a[:, j, :], x1n[:, j, :], **kw)
        nc.tensor.matmul(rowl_p[:], w1a[:, j, :], x1nl[:, j, :], **kw)
        nc.tensor.matmul(colD_p[:], w1c[:, j, :], x1n[:, j, :], **kw)
    colD = PAIRP.tile([128, L], f16, bufs=1)
    nc.scalar.activation(out=colD[:], in_=colD_p[:], func=AF.Copy)
    rl32 = PAIRP.tile([64, 96], f32, bufs=1)
    nc.scalar.activation(out=rl32[:], in_=rowl_p[:], func=AF.Copy)
    rowv = PAIRP.tile([128, NBLK], f32, bufs=1)
    dma(out=rowv[0:64, :], in_=rl32[:, 0::2])
    dma(out=rowv[64:128, :], in_=rl32[:, 1::2])

    # pair1 stats (exact, local): mean/var = row stats + col stats
    strc = ST.tile([64, 6], f32, tag="strc")
    mvp1 = ST.tile([64, 2], f32, tag="mvp1")
    mvc1 = ST.tile([64, 2], f32, tag="mvc1")
    nc.vector.bn_stats(out=strc[:], in_=rowg_p[:, :])
    nc.vector.bn_aggr(out=mvp1[:], in_=strc[:])
    nc.vector.bn_stats(out=strc[:], in_=colD[0:64, :])
    nc.vector.bn_aggr(out=mvc1[:], in_=strc[:])
    nc.vector.tensor_tensor(out=mvp1[:], in0=mvp1[:], in1=mvc1[:], op=OP.add)
    # A1 = g1*rstd, B1' = mean*A1 - b1  (no AR, no phase combine needed)
    ab64 = ST.tile([64, 2], f32, tag="ab64")
    varm = ST.tile([64, 1], f32, tag="varm")
    nc.scalar.activation(out=varm[:], in_=mvp1[:, 1:2], func=AF.Sqrt,
                         bias=eps[0:64, 0:1])
    nc.vector.reciprocal(out=varm[:], in_=varm[:])
    nc.vector.tensor_tensor(out=ab64[:, 0:1], in0=gpair[:, 0:1], in1=varm[:],
                            op=OP.mult)
    nc.vector.scalar_tensor_tensor(out=ab64[:, 1:2], in0=mvp1[:, 0:1],
                                   scalar=ab64[:, 0:1], in1=bpair[:, 0:1],
                                   op0=OP.mult, op1=OP.subtract)
    pd1 = PSS.tile([128, 2], f32, tag="pd")
    nc.tensor.matmul(pd1[:], eyeD[:, :], ab64[:, :], start=True, stop=True)
    ab1 = ST.tile([128, 2], f32, tag="ab")
    nc.vector.tensor_copy(out=ab1[:], in_=pd1[:])
    # Rv = A1*rowv - B1' ; p1 = lrelu(colD*A1 + Rv)
    Rv = ST.tile([128, NBLK], f32, tag="Bm")
    nc.vector.scalar_tensor_tensor(out=Rv[:], in0=rowv[:], scalar=ab1[:, 0:1],
                                   in1=_bcast_b1(ab1), op0=OP.mult,
                                   op1=OP.subtract)
    p1v = _padview(bufA, PB0, PB1)
    colD_bc = bass.AP(tensor=colD[:, :].tensor, offset=colD[:, :].offset,
                      ap=[colD[:, :].ap[0], [0, PB1 - PB0], [1, 384]])
    nc.vector.scalar_tensor_tensor(out=p1v, in0=colD_bc, scalar=ab1[:, 0:1],
                                   in1=_bcast(Rv, PB0, PB1),
                                   op0=OP.mult, op1=OP.add)
    nc.scalar.activation(out=p1v, in_=p1v, func=AF.Lrelu, alpha=0.01)

    # ---------------- pair2: scale W2 by rstd(x2), matmul pass ----------------
    w2a = PAIRP.tile([128, 64], f16, bufs=1)
    w2b = PAIRP.tile([128, 64], f16, bufs=1)
    dma(out=w2a[:], in_=ten["w2a_e"].ap())
    dma(out=w2b[:], in_=ten["w2b_e"].ap())
    # rstd for both groups from armx [128,4] = sum over cores [mA,qA,mB,qB]
    mg = ST.tile([128, 2], f32, tag="mg")
    vg = ST.tile([128, 2], f32, tag="vg")
    nc.vector.tensor_scalar(out=mg[:], in0=armx[:, 0::2], scalar1=0.125,
                            scalar2=None, op0=OP.mult)
    nc.vector.tensor_scalar(out=vg[:], in0=armx[:, 1::2], scalar1=0.125,
                            scalar2=None, op0=OP.mult)
    nc.vector.scalar_tensor_tensor(out=vg[:, 0:1], in0=mg[:, 0:1],
                                   scalar=mg[:, 0:1], in1=vg[:, 0:1],
                                   op0=OP.mult, op1=OP.subtract)  # m^2-q = -var
    nc.vector.scalar_tensor_tensor(out=vg[:, 1:2], in0=mg[:, 1:2],
                                   scalar=mg[:, 1:2], in1=vg[:, 1:2],
                                   op0=OP.mult, op1=OP.subtract)
    nc.scalar.activation(out=vg[:], in_=vg[:], func=AF.Sqrt, scale=-1.0,
                         bias=eps[:, 0:1])
    nc.vector.reciprocal(out=vg[:], in_=vg[:])   # rstd [128,2]
    w2as = PAIRP.tile([128, 64], f16, bufs=1)
    w2bs = PAIRP.tile([128, 64], f16, bufs=1)
    nc.vector.tensor_scalar(out=w2as[:], in0=w2a[:], scalar1=vg[:, 0:1],
                            scalar2=None, op0=OP.mult)
    nc.vector.tensor_scalar(out=w2bs[:], in0=w2b[:], scalar1=vg[:, 1:2],
                            scalar2=None, op0=OP.mult)

    for ci in list(range(5, 17)) + list(range(0, 5)) + list(range(17, 22)):
        xa = PAIRP.tile([128, 4 * 384], f16, tag="xa", name=f"xam{ci}")
        xb = PAIRP.tile([82, 4 * 384], f16, tag="xb", name=f"xbm{ci}")
        dma(out=xa[:].rearrange("p (r l) -> p r l", l=L),
            in_=ten["x2_e"].ap()[0:128, 4 + 4 * ci:8 + 4 * ci, :])
        nc.gpsimd.dma_start(out=xb[:].rearrange("p (r l) -> p r l", l=L),
                            in_=ten["x2_e"].ap()[128:210, 4 + 4 * ci:8 + 4 * ci, :])
        for t in range(2):
            b = PB0 + 2 * ci + t
            pp = PS.tile([128, 392], f32, tag="pp")
            er, od = 2 * t, 2 * t + 1
            nc.tensor.matmul(pp[0:64, 0:384], w2as[:, :],
                             xa[:, er * 384:(er + 1) * 384], start=True, stop=False)
            nc.tensor.matmul(pp[0:64, 0:384], w2bs[0:82, :],
                             xb[0:82, er * 384:(er + 1) * 384], start=False, stop=True)
            nc.tensor.matmul(pp[64:128, 0:384], w2as[:, :],
                             xa[:, od * 384:(od + 1) * 384], start=True, stop=False,
                             tile_position=(0, 64))
            nc.tensor.matmul(pp[64:128, 0:384], w2bs[0:82, :],
                             xb[0:82, od * 384:(od + 1) * 384], start=False, stop=True,
                             tile_position=(0, 64))
            evict(pp, b, lo=0)
        if ci == 16:  # owned blocks [12,36) done -> kick stats+AR early
            ms2 = stats_owned()
            arm2 = all_reduce(ms2, 2)
            ab2 = stats_to_ab(arm2, gpair[:, 1:2], bpair[:, 1:2])
    norm_tail(ab2, bufB, PB0, PB1)
    PAIRP.release()
    WP = tc.alloc_tile_pool(name="respool", bufs=2)

    # ---------------- pair3 into bufC ----------------
    def p3_group(g0, g1):
        pps = []
        for b in range(g0, g1):
            pp = PS.tile([128, 392], f32, tag="pp", name=f"pp3_{b}")
            pps.append(pp)
            nc.tensor.matmul(pp[:], w3sb[:, 0, :],
                             bufA[:, b * WPAD:(b + 1) * WPAD], start=True, stop=False)
            nc.tensor.matmul(pp[:], w3sb[:, 1, :],
                             bufB[:, b * WPAD:(b + 1) * WPAD], start=False, stop=True)
        for i, b in enumerate(range(g0, g1)):
            evict(pps[i], b)
    for g0 in range(OB0, OB1, 6):
        p3_group(g0, min(g0 + 6, OB1))
    ms3 = stats_owned()
    arm3 = all_reduce(ms3, 2)
    ab3 = stats_to_ab(arm3, gpair[:, 2:3], bpair[:, 2:3])
    for g0 in range(PB0, OB0, 6):
        p3_group(g0, min(g0 + 6, OB0))
    for g0 in range(OB1, PB1, 6):
        p3_group(g0, min(g0 + 6, PB1))
    norm_tail(ab3, bufC, PB0, PB1)

    # ---------------- ResNet: 5 blocks x 2 convs ----------------
    cur, tmp, nxt = bufC, bufA, bufB
    for q in range(10):
        d = DILS[q]
        b0, b1 = B0[q], B1[q]
        IN = cur if q % 2 == 0 else tmp
        TGT = tmp if q % 2 == 0 else nxt
        wf = WP.tile([128, 9, 128], f16, tag="wf")
        dma(out=wf[:], in_=ten["wf_e"].ap()[q].rearrange("t k m -> k t m"))
        if d == 1:
            wh = WP.tile([128, 6, 64], f16, tag="wh")
            dma(out=wh[:], in_=ten["wh_e"].ap()[q].rearrange("t k m -> k t m"))

        # taps: list of (kind, lhsT-getter, rhs partition range, delta, tilepos, outslice)
        taps = []
        if d != 1:
            for ki in range(3):
                for kj in range(3):
                    dd = (ki - 1) * (d // 2) * WPAD + (kj - 1) * d
                    taps.append(("f", ki * 3 + kj, dd))
        else:
            for kj in range(3):
                taps.append(("f", 3 + kj, kj - 1))
            for ki in (0, 2):
                for kj in range(3):
                    t = (0 if ki == 0 else 3) + kj
                    d0 = (-WPAD if ki == 0 else 0) + (kj - 1)
                    d1_ = (0 if ki == 0 else WPAD) + (kj - 1)
                    taps.append(("h", t, (d0, d1_)))
        ntap = len(taps)

        def do_group(g0, g1):
            pps = {b: PS.tile([128, 392], f32, tag="pp", name=f"pp_{q}_{b}")
                   for b in range(g0, g1)}
            for it, (kind, t, dd) in enumerate(taps):
                first, last = it == 0, it == ntap - 1
                for b in range(g0, g1):
                    pp = pps[b]
                    base = b * WPAD
                    if kind == "f":
                        nc.tensor.matmul(pp[:], wf[:, t, :],
                                         IN[:, base + dd:base + dd + WPAD],
                                         start=first, stop=False,
                                         skip_group_check=True)
                    else:
                        d0, d1_ = dd
                        nc.tensor.matmul(pp[0:64, :], wh[64:128, t, :],
                                         IN[64:128, base + d0:base + d0 + WPAD],
                                         start=False, stop=False,
                                         tile_position=(64, 0),
                                         skip_group_check=True)
                        nc.tensor.matmul(pp[64:128, :], wh[0:64, t, :],
                                         IN[0:64, base + d1_:base + d1_ + WPAD],
                                         start=False, stop=last,
                                         tile_position=(0, 64),
                                         skip_group_check=True)
            for b in range(g0, g1):
                evict(pps[b], b)

        # owned blocks first: stats+AR issue early and hide under halo groups
        for g0 in range(OB0, OB1, 6):
            do_group(g0, min(g0 + 6, OB1))
        ms = stats_owned()
        arm = all_reduce(ms, 2)
        ab = stats_to_ab(arm, gres[:, q:q + 1], bres[:, q:q + 1])
        for g0 in range(b0, OB0, 6):
            do_group(g0, min(g0 + 6, OB0))
        for g0 in range(OB1, b1, 6):
            do_group(g0, min(g0 + 6, b1))
        norm_tail(ab, TGT, b0, b1, res=(cur if q % 2 == 1 else None))
        if q % 2 == 1:
            cur, nxt = nxt, cur

    # ---------------- output: owned blocks, fp32, de-phase ----------------
    for hf in range(2):
        ob0 = OB0 + 12 * hf
        o32 = WP.tile([128, 12 * 384], f32, tag="o32", bufs=1, name=f"o32_{hf}")
        nc.vector.tensor_copy(out=o32[:].rearrange("p (b l) -> p b l", l=384),
                              in_=_padview(cur, ob0, ob0 + 12))
        oap = ten["out_e"].ap()
        dma(out=oap[:, 24 * hf + 0:24 * hf + 24:2, :],
            in_=o32[0:64, :].rearrange("p (b l) -> p b l", l=384))
        dma(out=oap[:, 24 * hf + 1:24 * hf + 24:2, :],
            in_=o32[64:128, :].rearrange("p (b l) -> p b l", l=384))
    WP.release()
    ctx.close()


def _bcast_b1(ab1):
    base = ab1[:, 1:2]
    return bass.AP(tensor=base.tensor, offset=base.offset,
                   ap=[base.ap[0], [0, NBLK]])


# ======================= host side =======================

_NC = None


def _get_nc():
    global _NC
    if _NC is None:
        _NC = build()
    return _NC


def _prep(x_1d, x_2d, W1, g1, b1, W2, g2, b2, W3, g3, b3,
          res_w, res_b, res_g, res_beta):
    n16 = lambda a: np.ascontiguousarray(np.asarray(a, np.float32).astype(np.float16))
    n32 = lambda a: np.ascontiguousarray(np.asarray(a, np.float32))

    x1 = np.zeros((896, L), np.float16)
    x1[:D1] = n16(x_1d[0])
    x2f = n16(x_2d[0])                      # [210, 384, 384]

    W1 = np.asarray(W1, np.float32)
    w1a = np.zeros((7, 128, 64), np.float16)
    w1c = np.zeros((7, 128, 128), np.float16)
    for a in range(7):
        r0, r1 = a * 128, min((a + 1) * 128, D1)
        w1a[a, :r1 - r0, :] = W1[:, :D1][:, r0:r1].T.astype(np.float16)
        wb = W1[:, D1:][:, r0:r1].T.astype(np.float16)
        w1c[a, :r1 - r0, 0:64] = wb
        w1c[a, :r1 - r0, 64:128] = wb
    W2 = np.asarray(W2, np.float32)
    w2a = np.ascontiguousarray(W2[:, 0:128].T.astype(np.float16))
    w2b = np.zeros((128, 64), np.float16)
    w2b[0:82] = W2[:, 128:210].T.astype(np.float16)
    W3 = np.asarray(W3, np.float32)
    e2 = np.eye(2, dtype=np.float32)
    w3 = np.stack([np.kron(e2, W3[:, :CH].T), np.kron(e2, W3[:, CH:].T)])
    w3 = w3.astype(np.float16)
    wf = np.zeros((10, 9, 128, 128), np.float16)
    wh = np.zeros((10, 6, 128, 64), np.float16)
    res_w = np.asarray(res_w, np.float32)
    for q in range(10):
        w = res_w[q // 2, q % 2]            # [O, I, 3, 3]
        dq = DILS[q]
        for ki in range(3):
            for kj in range(3):
                tT = w[:, :, ki, kj].T.astype(np.float16)   # [I, O]
                if dq != 1 or ki == 1:
                    wf[q, ki * 3 + kj] = np.kron(e2, tT)
                else:
                    t = (0 if ki == 0 else 3) + kj
                    wh[q, t, 0:64] = tT
                    wh[q, t, 64:128] = tT
    eyeS = np.zeros((128, 64), np.float32)
    eyeD = np.zeros((64, 128), np.float32)
    for m in range(64):
        eyeS[m, m] = eyeS[m + 64, m] = 1.0 / 16.0
        eyeD[m, m] = eyeD[m, m + 64] = 1.0
    gpair = n32(np.stack([g1, g2, g3], 1))
    bpair = n32(np.stack([b1, b2, b3], 1))
    gresv = n32(np.asarray(res_g, np.float32).reshape(10, CH).T)
    bresv = n32(np.asarray(res_beta, np.float32).reshape(10, CH).T)

    common = dict(x1=x1, w1a=w1a, w1c=w1c, w2a=w2a, w2b=w2b, w3=w3, wf=wf,
                  wh=wh, eyeS=eyeS, eyeD=eyeD, gpair=gpair, bpair=bpair,
                  gres=gresv, bres=bresv)
    in_maps = []
    for c in range(NCOR):
        r0 = 48 * c - 24
        x2s = np.zeros((D2, 96, L), np.float16)
        lo, hi = max(0, r0), min(L, r0 + 96)
        x2s[:, lo - r0:hi - r0, :] = x2f[:, lo:hi, :]
        x1l = np.zeros((896, 96), np.float16)
        x1l[:, lo - r0:hi - r0] = x1[:, lo:hi]
        msk = np.zeros((128, NBLK), np.float32)
        for p in range(2):
            for b in range(NBLK):
                r = r0 + 2 * b + p
                if 0 <= r < L:
                    msk[64 * p:64 * (p + 1), b] = 1.0
        in_maps.append(dict(common, x2=x2s, x1loc=x1l, mask=msk))
    return in_maps


def _run(inputs, trace=False):
    nc = _get_nc()
    in_maps = _prep(**inputs)
    res = run_bass_kernel_spmd(nc, in_maps, core_ids=list(range(NCOR)),
                               trace=trace)
    out = np.empty((1, CH, L, L), np.float32)
    for c in range(NCOR):
        out[0, :, 48 * c:48 * (c + 1), :] = res.results[c]["out"]
    return out, res


def kernel(**inputs):
    out, _ = _run(inputs, trace=False)
    return out


# revision 11
# speedup vs baseline: 1.2075x; 1.0552x over previous
"""nn_Intra_ResNet on 8 TRN2 NeuronCores (Bass/Tile, SPMD).

Row-sharded 8-way (48 rows/core) with 20-row halo recompute (no halo
exchange). Activations live in SBUF phase-packed: partition = ch + 64*(row
parity), column = block*392 + 4 + j  (a "block" is a pair of image rows,
392 = 384 + 2*4 zero pad columns). Every 3x3 dilated conv tap is then a
K=128/M=128 fp16 matmul at a column offset (even-d taps and d=1 center
row), or a pair of concurrent 64x64 quadrant matmuls via tile_position
(d=1 phase-flip taps). InstanceNorm: per-partition bn_stats + one [128,2]
fp32 AllReduce per norm (13 total), then a fused scale/bias (+mask) +
LeakyReLU applied with 3 big strided ops (gpsimd z-pass, scalar Lrelu,
vector residual-add).
"""
import sys

for _p in ("/opt/trn_rl_repo",):
    if _p not in sys.path:
        sys.path.insert(0, _p)

import numpy as np

import concourse.bass as bass
import concourse.tile as tile
from concourse import bacc, mybir
from concourse.bass_utils import run_bass_kernel_spmd

f16 = mybir.dt.float16
f32 = mybir.dt.float32
AF = mybir.ActivationFunctionType
OP = mybir.AluOpType

NCOR = 8
L = 384
CH = 64
D1 = 788
D2 = 210
EPS = 1e-5
DILS = [1, 1, 2, 2, 4, 4, 2, 2, 1, 1]
OWN = 48          # rows per core
NBLK = 48         # buffer blocks (96 rows incl 2 guard blocks/side)
WPAD = 392        # padded row width (4 | 384 | 4)
PADL = 4
RG = [list(range(NCOR))]

# halo rows needed before conv q; conv q computes local rows [24-H[q+1], 72+H[q+1])
H = [sum(DILS[i:]) for i in range(11)]          # [20,19,18,16,14,10,6,4,2,1,0]
B0 = [(24 - H[q + 1]) // 2 for q in range(10)]  # [2,3,4,5,7,9,10,11,11,12]
B1 = [(72 + H[q + 1] + 1) // 2 for q in range(10)]
PB0, PB1 = 2, 46   # pair-stage computed blocks
OB0, OB1 = 12, 36  # owned blocks


def _padview(buf, b0, b1):
    """[128, b1-b0, 384] view of padded-layout buffer (data cols only)."""
    base = buf[:, 0:1]
    return bass.AP(tensor=base.tensor, offset=base.offset + b0 * WPAD + PADL,
                   ap=[base.ap[0], [WPAD, b1 - b0], [1, 384]])


def _bcast(t2d, b0, b1):
    """[128, b1-b0, 384] broadcast view of a [128, NBLK] per-block tile."""
    base = t2d[:, b0:b1]
    return bass.AP(tensor=base.tensor, offset=base.offset,
                   ap=[base.ap[0], base.ap[1], [0, 384]])


def _rawview(raw, b0, b1):
    return raw[:, b0 * 384:b1 * 384].rearrange("p (b l) -> p b l", l=384)


def build():
    nc = bacc.Bacc("TRN2", target_bir_lowering=False, debug=False,
                   num_devices=NCOR)
    ein = dict(kind="ExternalInput")
    x1_e = nc.dram_tensor("x1", [896, L], f16, **ein)
    x1l_e = nc.dram_tensor("x1loc", [896, 96], f16, **ein)
    x2_e = nc.dram_tensor("x2", [D2, 96, L], f16, **ein)
    mask_e = nc.dram_tensor("mask", [128, NBLK], f32, **ein)
    w1a_e = nc.dram_tensor("w1a", [7, 128, 64], f16, **ein)
    w1c_e = nc.dram_tensor("w1c", [7, 128, 128], f16, **ein)
    w2a_e = nc.dram_tensor("w2a", [128, 64], f16, **ein)
    w2b_e = nc.dram_tensor("w2b", [128, 64], f16, **ein)
    w3_e = nc.dram_tensor("w3", [2, 128, 128], f16, **ein)
    wf_e = nc.dram_tensor("wf", [10, 9, 128, 128], f16, **ein)
    wh_e = nc.dram_tensor("wh", [10, 6, 128, 64], f16, **ein)
    eyeS_e = nc.dram_tensor("eyeS", [128, 64], f32, **ein)
    eyeD_e = nc.dram_tensor("eyeD", [64, 128], f32, **ein)
    gp_e = nc.dram_tensor("gpair", [64, 3], f32, **ein)
    bp_e = nc.dram_tensor("bpair", [64, 3], f32, **ein)
    gr_e = nc.dram_tensor("gres", [64, 10], f32, **ein)
    br_e = nc.dram_tensor("bres", [64, 10], f32, **ein)
    out_e = nc.dram_tensor("out", [CH, OWN, L], f32, kind="ExternalOutput")

    with tile.TileContext(nc) as tc:
        _body(nc, tc, locals())
    nc.compile()
    return nc


def _body(nc, tc, ten):
    from contextlib import ExitStack
    ctx = ExitStack()
    P = ctx.enter_context(tc.tile_pool(name="persist", bufs=1))
    ST = ctx.enter_context(tc.tile_pool(name="stats", bufs=2))
    PS = ctx.enter_context(tc.tile_pool(name="psum", bufs=6, space="PSUM"))
    PSS = ctx.enter_context(tc.tile_pool(name="psmall", bufs=1, space="PSUM"))
    DR = ctx.enter_context(tc.tile_pool(name="dram", bufs=2, space="DRAM"))

    dma = nc.sync.dma_start

    bufA = P.tile([128, NBLK * WPAD], f16)
    bufB = P.tile([128, NBLK * WPAD], f16)
    bufC = P.tile([128, NBLK * WPAD], f16)
    raw = P.tile([128, NBLK * 384], f16)
    mask = P.tile([128, NBLK], f32)
    eyeS = P.tile([128, 64], f32)
    eyeD = P.tile([64, 128], f32)
    gpair = P.tile([64, 3], f32)
    bpair = P.tile([64, 3], f32)
    gres = P.tile([64, 10], f32)
    bres = P.tile([64, 10], f32)
    w3sb = P.tile([128, 2, 128], f16)
    eps = P.tile([128, 1], f32)

    dma(out=mask[:], in_=ten["mask_e"].ap())
    dma(out=eyeS[:], in_=ten["eyeS_e"].ap())
    dma(out=eyeD[:], in_=ten["eyeD_e"].ap())
    dma(out=gpair[:], in_=ten["gp_e"].ap())
    dma(out=bpair[:], in_=ten["bp_e"].ap())
    dma(out=gres[:], in_=ten["gr_e"].ap())
    dma(out=bres[:], in_=ten["br_e"].ap())
    dma(out=w3sb[:], in_=ten["w3_e"].ap().rearrange("t k m -> k t m"))
    nc.vector.memset(eps[:], float(EPS))

    nc.vector.memset(bufA[:], 0.0)
    nc.gpsimd.memset(bufB[:], 0.0)
    nc.vector.memset(bufC[:], 0.0)

    def all_reduce(ms, width):
        arin = DR.tile([128, width], f32, tag="arin")
        arout = DR.tile([128, width], f32, tag="arout", addr_space="Shared")
        dma(out=arin[:], in_=ms[:, 0:width])
        nc.gpsimd.collective_compute(
            "AllReduce", OP.add, replica_groups=RG,
            ins=[arin[:].opt()], outs=[arout[:].opt()])
        armv = ST.tile([128, width], f32, tag="armv")
        dma(out=armv[:, 0:width], in_=arout[:])
        return armv

    def stats_owned():
        """bn_stats over owned blocks of raw -> [128,2] sbuf [mean, msq]."""
        st6 = ST.tile([128, 18, 6], f32, tag="st6")
        for i in range(18):
            nc.vector.bn_stats(out=st6[:, i, :],
                               in_=raw[:, 4608 + 512 * i:4608 + 512 * (i + 1)])
        mv = ST.tile([128, 2], f32, tag="mv")
        nc.vector.bn_aggr(out=mv[:], in_=st6[:])
        ms = ST.tile([128, 2], f32, tag="ms")
        nc.vector.tensor_copy(out=ms[:, 0:1], in_=mv[:, 0:1])
        nc.vector.scalar_tensor_tensor(out=ms[:, 1:2], in0=mv[:, 0:1],
                                       scalar=mv[:, 0:1], in1=mv[:, 1:2],
                                       op0=OP.mult, op1=OP.add)
        return ms

    def stats_to_ab(armv, g64, b64):
        """armv [128,2] (sum over 8 cores of per-partition [mean,msq]) ->
        ab [128,2] f32: col0 A = g*rstd, col1 B' = gmean*A - beta."""
        pm = PSS.tile([64, 2], f32, tag="pm")
        nc.tensor.matmul(pm[:], eyeS[:, :], armv[:, 0:2], start=True, stop=True)
        pmv = ST.tile([64, 2], f32, tag="pmv")
        nc.scalar.activation(out=pmv[:], in_=pm[:], func=AF.Copy)
        ab64 = ST.tile([64, 2], f32, tag="ab64")
        varm = ST.tile([64, 1], f32, tag="varm")
        nc.vector.scalar_tensor_tensor(out=varm[:], in0=pmv[:, 0:1],
                                       scalar=pmv[:, 0:1], in1=pmv[:, 1:2],
                                       op0=OP.mult, op1=OP.subtract)  # -var
        nc.scalar.activation(out=varm[:], in_=varm[:], func=AF.Sqrt,
                             scale=-1.0, bias=eps[0:64, 0:1])
        nc.vector.reciprocal(out=varm[:], in_=varm[:])                # rstd
        nc.vector.tensor_tensor(out=ab64[:, 0:1], in0=g64, in1=varm[:],
                                op=OP.mult)                           # A
        nc.vector.scalar_tensor_tensor(out=ab64[:, 1:2], in0=pmv[:, 0:1],
                                       scalar=ab64[:, 0:1], in1=b64,
                                       op0=OP.mult, op1=OP.subtract)  # B'
        pd = PSS.tile([128, 2], f32, tag="pd")
        nc.tensor.matmul(pd[:], eyeD[:, :], ab64[:, :], start=True, stop=True)
        ab = ST.tile([128, 2], f32, tag="ab")
        nc.vector.tensor_copy(out=ab[:], in_=pd[:])
        return ab

    def norm_tail(ab, tgt, b0, b1, res=None, nch=4):
        """Chunked z = A*raw - B'*mask (vector) + Lrelu in place (scalar)
        + res (gpsimd). Owned chunks first so the next conv's inner-owned
        matmuls can start before halo chunks normalize."""
        Bm = ST.tile([128, NBLK], f32, tag="Bm")
        nc.vector.tensor_scalar(out=Bm[:], in0=mask[:], scalar1=ab[:, 1:2],
                                scalar2=None, op0=OP.mult)
        spans = [(OB0, (OB0 + OB1) // 2), ((OB0 + OB1) // 2, OB1)]
        if b0 < OB0:
            spans.append((b0, OB0))
        if b1 > OB1:
            spans.append((OB1, b1))
        for c0, c1 in spans:
            if c1 <= c0:
                continue
            tv = _padview(tgt, c0, c1)
            nc.vector.scalar_tensor_tensor(out=tv, in0=_rawview(raw, c0, c1),
                                           scalar=ab[:, 0:1],
                                           in1=_bcast(Bm, c0, c1),
                                           op0=OP.mult, op1=OP.subtract)
            nc.scalar.activation(out=tv, in_=tv, func=AF.Lrelu, alpha=0.01)
            if res is not None:
                nc.gpsimd.tensor_tensor(out=tv, in0=tv,
                                        in1=_padview(res, c0, c1), op=OP.add)

    def evict(pp, b, n=392, lo=PADL):
        """psum block -> raw, masked per block (ACT: Copy with scale AP)."""
        nc.scalar.activation(out=raw[:, b * 384:(b + 1) * 384],
                             in_=pp[:, lo:lo + 384], func=AF.Copy,
                             scale=mask[:, b:b + 1])

    # ---------------- x2 stats pass (owned rows) + AR ----------------
    PAIRP = tc.alloc_tile_pool(name="pairp", bufs=2)
    st6a = PAIRP.tile([128, 36, 6], f32, bufs=1)
    st6b = PAIRP.tile([82, 36, 6], f32, bufs=1)
    qs = [nc.sync, nc.gpsimd, nc.scalar]
    for t in range(12):
        xa = PAIRP.tile([128, 4 * 384], f16, tag="xa", name=f"xas{t}")
        xb = PAIRP.tile([82, 4 * 384], f16, tag="xb", name=f"xbs{t}")
        qs[t % 3].dma_start(out=xa[:].rearrange("p (r l) -> p r l", l=L),
                            in_=ten["x2_e"].ap()[0:128, 24 + 4 * t:28 + 4 * t, :])
        qs[(t + 1) % 3].dma_start(out=xb[:].rearrange("p (r l) -> p r l", l=L),
                                  in_=ten["x2_e"].ap()[128:210, 24 + 4 * t:28 + 4 * t, :])
        for i in range(3):
            nc.vector.bn_stats(out=st6a[:, 3 * t + i, :],
                               in_=xa[:, 512 * i:512 * (i + 1)])
            nc.vector.bn_stats(out=st6b[:, 3 * t + i, :],
                               in_=xb[:, 512 * i:512 * (i + 1)])
    mvx = ST.tile([128, 4], f32, tag="mvx")
    nc.vector.memset(mvx[:], 0.0)
    mva = ST.tile([128, 2], f32, tag="mv")
    nc.vector.bn_aggr(out=mva[:], in_=st6a[:])
    mvb = ST.tile([82, 2], f32, tag="mvb")
    nc.vector.bn_aggr(out=mvb[:], in_=st6b[:])
    nc.vector.tensor_copy(out=mvx[:, 0:1], in_=mva[:, 0:1])
    nc.vector.scalar_tensor_tensor(out=mvx[:, 1:2], in0=mva[:, 0:1],
                                   scalar=mva[:, 0:1], in1=mva[:, 1:2],
                                   op0=OP.mult, op1=OP.add)
    nc.vector.tensor_copy(out=mvx[0:82, 2:3], in_=mvb[:, 0:1])
    nc.vector.scalar_tensor_tensor(out=mvx[0:82, 3:4], in0=mvb[:, 0:1],
                                   scalar=mvb[:, 0:1], in1=mvb[:, 1:2],
                                   op0=OP.mult, op1=OP.add)
    armx = all_reduce(mvx, 4)   # AR#1 (in flight during pair1)

    # ---------------- x1 norm + pair1 into bufA ----------------
    x1sb = PAIRP.tile([128, 7, L], f16, bufs=1)
    x1n = PAIRP.tile([128, 7, L], f16, bufs=1)
    x1ls = PAIRP.tile([128, 7, 96], f16, bufs=1)
    x1nl = PAIRP.tile([128, 7, 96], f16, bufs=1)
    w1a = PAIRP.tile([128, 7, 64], f16, bufs=1)
    w1c = PAIRP.tile([128, 7, 128], f16, bufs=1)
    dma(out=x1sb[:], in_=ten["x1_e"].ap().rearrange("(a p) l -> p a l", p=128))
    dma(out=x1ls[:], in_=ten["x1l_e"].ap().rearrange("(a p) l -> p a l", p=128))
    dma(out=w1a[:], in_=ten["w1a_e"].ap().rearrange("a k m -> k a m"))
    dma(out=w1c[:], in_=ten["w1c_e"].ap().rearrange("a k m -> k a m"))
    st1 = ST.tile([128, 6], f32, tag="st1")
    mv1 = ST.tile([128, 7, 2], f32, tag="mv1")
    for j in range(7):
        nc.vector.bn_stats(out=st1[:], in_=x1sb[:, j, :])
        nc.vector.bn_aggr(out=mv1[:, j, :], in_=st1[:])
    rst1 = ST.tile([128, 7], f32, tag="rst1")
    nc.scalar.activation(out=rst1[:], in_=mv1[:, :, 1], func=AF.Sqrt,
                         bias=eps[:, 0:1])
    nc.vector.reciprocal(out=rst1[:], in_=rst1[:])
    for j in range(7):
        nc.vector.tensor_scalar(out=x1n[:, j, :], in0=x1sb[:, j, :],
                                scalar1=mv1[:, j, 0:1], scalar2=rst1[:, j:j + 1],
                                op0=OP.subtract, op1=OP.mult)
        nc.vector.tensor_scalar(out=x1nl[:, j, :], in0=x1ls[:, j, :],
                                scalar1=mv1[:, j, 0:1], scalar2=rst1[:, j:j + 1],
                                op0=OP.subtract, op1=OP.mult)

    rowg_p = PS.tile([64, L], f32, tag="pp")
    rowl_p = PS.tile([64, 96], f32, tag="pp")
    colD_p = PS.tile([128, L], f32, tag="pp")
    for j in range(7):
        kw = dict(start=(j == 0), stop=(j == 6))
        nc.tensor.matmul(rowg_p[:], w1a[:, j, :], x1n[:, j, :], **kw)
        nc.tensor.matmul(rowl_p[:], w1a[:, j, :], x1nl[:, j, :], **kw)
        nc.tensor.matmul(colD_p[:], w1c[:, j, :], x1n[:, j, :], **kw)
    colD = PAIRP.tile([128, L], f16, bufs=1)
    nc.scalar.activation(out=colD[:], in_=colD_p[:], func=AF.Copy)
    rl32 = PAIRP.tile([64, 96], f32, bufs=1)
    nc.scalar.activation(out=rl32[:], in_=rowl_p[:], func=AF.Copy)
    rowv = PAIRP.tile([128, NBLK], f32, bufs=1)
    dma(out=rowv[0:64, :], in_=rl32[:, 0::2])
    dma(out=rowv[64:128, :], in_=rl32[:, 1::2])

    # pair1 stats (exact, local): mean/var = row stats + col stats
    strc = ST.tile([64, 6], f32, tag="strc")
    mvp1 = ST.tile([64, 2], f32, tag="mvp1")
    mvc1 = ST.tile([64, 2], f32, tag="mvc1")
    nc.vector.bn_stats(out=strc[:], in_=rowg_p[:, :])
    nc.vector.bn_aggr(out=mvp1[:], in_=strc[:])
    nc.vector.bn_stats(out=strc[:], in_=colD[0:64, :])
    nc.vector.bn_aggr(out=mvc1[:], in_=strc[:])
    nc.vector.tensor_tensor(out=mvp1[:], in0=mvp1[:], in1=mvc1[:], op=OP.add)
    # A1 = g1*rstd, B1' = mean*A1 - b1  (no AR, no phase combine needed)
    ab64 = ST.tile([64, 2], f32, tag="ab64")
    varm = ST.tile([64, 1], f32, tag="varm")
    nc.scalar.activation(out=varm[:], in_=mvp1[:, 1:2], func=AF.Sqrt,
                         bias=eps[0:64, 0:1])
    nc.vector.reciprocal(out=varm[:], in_=varm[:])
    nc.vector.tensor_tensor(out=ab64[:, 0:1], in0=gpair[:, 0:1], in1=varm[:],
                            op=OP.mult)
    nc.vector.scalar_tensor_tensor(out=ab64[:, 1:2], in0=mvp1[:, 0:1],
                                   scalar=ab64[:, 0:1], in1=bpair[:, 0:1],
                                   op0=OP.mult, op1=OP.subtract)
    pd1 = PSS.tile([128, 2], f32, tag="pd")
    nc.tensor.matmul(pd1[:], eyeD[:, :], ab64[:, :], start=True, stop=True)
    ab1 = ST.tile([128, 2], f32, tag="ab")
    nc.vector.tensor_copy(out=ab1[:], in_=pd1[:])
    # Rv = A1*rowv - B1' ; p1 = lrelu(colD*A1 + Rv)
    Rv = ST.tile([128, NBLK], f32, tag="Bm")
    nc.vector.scalar_tensor_tensor(out=Rv[:], in0=rowv[:], scalar=ab1[:, 0:1],
                                   in1=_bcast_b1(ab1), op0=OP.mult,
                                   op1=OP.subtract)
    p1v = _padview(bufA, PB0, PB1)
    colD_bc = bass.AP(tensor=colD[:, :].tensor, offset=colD[:, :].offset,
                      ap=[colD[:, :].ap[0], [0, PB1 - PB0], [1, 384]])
    nc.vector.scalar_tensor_tensor(out=p1v, in0=colD_bc, scalar=ab1[:, 0:1],
                                   in1=_bcast(Rv, PB0, PB1),
                                   op0=OP.mult, op1=OP.add)
    nc.scalar.activation(out=p1v, in_=p1v, func=AF.Lrelu, alpha=0.01)

    # ---------------- pair2: scale W2 by rstd(x2), matmul pass ----------------
    w2a = PAIRP.tile([128, 64], f16, bufs=1)
    w2b = PAIRP.tile([128, 64], f16, bufs=1)
    dma(out=w2a[:], in_=ten["w2a_e"].ap())
    dma(out=w2b[:], in_=ten["w2b_e"].ap())
    # rstd for both groups from armx [128,4] = sum over cores [mA,qA,mB,qB]
    mg = ST.tile([128, 2], f32, tag="mg")
    vg = ST.tile([128, 2], f32, tag="vg")
    nc.vector.tensor_scalar(out=mg[:], in0=armx[:, 0::2], scalar1=0.125,
                            scalar2=None, op0=OP.mult)
    nc.vector.tensor_scalar(out=vg[:], in0=armx[:, 1::2], scalar1=0.125,
                            scalar2=None, op0=OP.mult)
    nc.vector.scalar_tensor_tensor(out=vg[:, 0:1], in0=mg[:, 0:1],
                                   scalar=mg[:, 0:1], in1=vg[:, 0:1],
                                   op0=OP.mult, op1=OP.subtract)  # m^2-q = -var
    nc.vector.scalar_tensor_tensor(out=vg[:, 1:2], in0=mg[:, 1:2],
                                   scalar=mg[:, 1:2], in1=vg[:, 1:2],
                                   op0=OP.mult, op1=OP.subtract)
    nc.scalar.activation(out=vg[:], in_=vg[:], func=AF.Sqrt, scale=-1.0,
                         bias=eps[:, 0:1])
    nc.vector.reciprocal(out=vg[:], in_=vg[:])   # rstd [128,2]
    w2as = PAIRP.tile([128, 64], f16, bufs=1)
    w2bs = PAIRP.tile([128, 64], f16, bufs=1)
    nc.vector.tensor_scalar(out=w2as[:], in0=w2a[:], scalar1=vg[:, 0:1],
                            scalar2=None, op0=OP.mult)
    nc.vector.tensor_scalar(out=w2bs[:], in0=w2b[:], scalar1=vg[:, 1:2],
                            scalar2=None, op0=OP.mult)

    for ci in list(range(5, 17)) + list(range(0, 5)) + list(range(17, 22)):
        xa = PAIRP.tile([128, 4 * 384], f16, tag="xa", name=f"xam{ci}")
        xb = PAIRP.tile([82, 4 * 384], f16, tag="xb", name=f"xbm{ci}")
        qs[ci % 3].dma_start(out=xa[:].rearrange("p (r l) -> p r l", l=L),
                             in_=ten["x2_e"].ap()[0:128, 4 + 4 * ci:8 + 4 * ci, :])
        qs[(ci + 1) % 3].dma_start(out=xb[:].rearrange("p (r l) -> p r l", l=L),
                                   in_=ten["x2_e"].ap()[128:210, 4 + 4 * ci:8 + 4 * ci, :])
        for t in range(2):
            b = PB0 + 2 * ci + t
            pp = PS.tile([128, 392], f32, tag="pp")
            er, od = 2 * t, 2 * t + 1
            nc.tensor.matmul(pp[0:64, 0:384], w2as[:, :],
                             xa[:, er * 384:(er + 1) * 384], start=True, stop=False)
            nc.tensor.matmul(pp[0:64, 0:384], w2bs[0:82, :],
                             xb[0:82, er * 384:(er + 1) * 384], start=False, stop=True)
            nc.tensor.matmul(pp[64:128, 0:384], w2as[:, :],
                             xa[:, od * 384:(od + 1) * 384], start=True, stop=False,
                             tile_position=(0, 64))
            nc.tensor.matmul(pp[64:128, 0:384], w2bs[0:82, :],
                             xb[0:82, od * 384:(od + 1) * 384], start=False, stop=True,
                             tile_position=(0, 64))
            evict(pp, b, lo=0)
        if ci == 16:  # owned blocks [12,36) done -> kick stats+AR early
            ms2 = stats_owned()
            arm2 = all_reduce(ms2, 2)
            ab2 = stats_to_ab(arm2, gpair[:, 1:2], bpair[:, 1:2])
    norm_tail(ab2, bufB, PB0, PB1)
    PAIRP.release()
    WP = tc.alloc_tile_pool(name="respool", bufs=2)

    # ---------------- pair3 into bufC ----------------
    def p3_group(g0, g1):
        pps = []
        for b in range(g0, g1):
            pp = PS.tile([128, 392], f32, tag="pp", name=f"pp3_{b}")
            pps.append(pp)
            nc.tensor.matmul(pp[:], w3sb[:, 0, :],
                             bufA[:, b * WPAD:(b + 1) * WPAD], start=True, stop=False)
            nc.tensor.matmul(pp[:], w3sb[:, 1, :],
                             bufB[:, b * WPAD:(b + 1) * WPAD], start=False, stop=True)
        for i, b in enumerate(range(g0, g1)):
            evict(pps[i], b)
    for g0 in range(OB0, OB1, 6):
        p3_group(g0, min(g0 + 6, OB1))
    ms3 = stats_owned()
    arm3 = all_reduce(ms3, 2)
    ab3 = stats_to_ab(arm3, gpair[:, 2:3], bpair[:, 2:3])
    for g0 in range(PB0, OB0, 6):
        p3_group(g0, min(g0 + 6, OB0))
    for g0 in range(OB1, PB1, 6):
        p3_group(g0, min(g0 + 6, PB1))
    norm_tail(ab3, bufC, PB0, PB1)

    # ---------------- ResNet: 5 blocks x 2 convs ----------------
    cur, tmp, nxt = bufC, bufA, bufB
    for q in range(10):
        d = DILS[q]
        b0, b1 = B0[q], B1[q]
        IN = cur if q % 2 == 0 else tmp
        TGT = tmp if q % 2 == 0 else nxt
        wf = WP.tile([128, 9, 128], f16, tag="wf")
        dma(out=wf[:], in_=ten["wf_e"].ap()[q].rearrange("t k m -> k t m"))
        if d == 1:
            wh = WP.tile([128, 6, 64], f16, tag="wh")
            dma(out=wh[:], in_=ten["wh_e"].ap()[q].rearrange("t k m -> k t m"))

        # taps: list of (kind, lhsT-getter, rhs partition range, delta, tilepos, outslice)
        taps = []
        if d != 1:
            for ki in range(3):
                for kj in range(3):
                    dd = (ki - 1) * (d // 2) * WPAD + (kj - 1) * d
                    taps.append(("f", ki * 3 + kj, dd))
        else:
            for kj in range(3):
                taps.append(("f", 3 + kj, kj - 1))
            for kj in range(3):
                # leftover phase-flip halves: p0-out reads phase1 @ -392,
                # p1-out reads phase0 @ +392 (concurrent quadrant pair)
                taps.append(("h", (kj, 3 + kj),
                             (-WPAD + kj - 1, WPAD + kj - 1)))
        ntap = len(taps)

        def do_group(g0, g1):
            pps = {b: PS.tile([128, 392], f32, tag="pp", name=f"pp_{q}_{b}")
                   for b in range(g0, g1)}
            for it, (kind, t, dd) in enumerate(taps):
                first, last = it == 0, it == ntap - 1
                for b in range(g0, g1):
                    pp = pps[b]
                    base = b * WPAD
                    if kind == "f":
                        nc.tensor.matmul(pp[:], wf[:, t, :],
                                         IN[:, base + dd:base + dd + WPAD],
                                         start=first, stop=False,
                                         skip_group_check=True)
                    else:
                        t0, t1 = t
                        d0, d1_ = dd
                        nc.tensor.matmul(pp[0:64, :], wh[64:128, t0, :],
                                         IN[64:128, base + d0:base + d0 + WPAD],
                                         start=False, stop=False,
                                         tile_position=(64, 0),
                                         skip_group_check=True)
                        nc.tensor.matmul(pp[64:128, :], wh[0:64, t1, :],
                                         IN[0:64, base + d1_:base + d1_ + WPAD],
                                         start=False, stop=last,
                                         tile_position=(0, 64),
                                         skip_group_check=True)
            for b in range(g0, g1):
                evict(pps[b], b)

        # inner-owned first (inputs need only the previous conv's owned
        # normalize chunks), then outer-owned, then stats+AR, then halo
        # groups (hide the AR), then normalize.
        for g0 in range(OB0 + 2, OB1 - 2, 6):
            do_group(g0, min(g0 + 6, OB1 - 2))
        do_group(OB0, OB0 + 2)
        do_group(OB1 - 2, OB1)
        ms = stats_owned()
        arm = all_reduce(ms, 2)
        ab = stats_to_ab(arm, gres[:, q:q + 1], bres[:, q:q + 1])
        for g0 in range(b0, OB0, 6):
            do_group(g0, min(g0 + 6, OB0))
        for g0 in range(OB1, b1, 6):
            do_group(g0, min(g0 + 6, b1))
        norm_tail(ab, TGT, b0, b1, res=(cur if q % 2 == 1 else None))
        if q % 2 == 1:
            cur, nxt = nxt, cur

    # ---------------- output: owned blocks, fp32, de-phase ----------------
    for hf in range(2):
        ob0 = OB0 + 12 * hf
        o32 = WP.tile([128, 12 * 384], f32, tag="o32", bufs=1, name=f"o32_{hf}")
        nc.vector.tensor_copy(out=o32[:].rearrange("p (b l) -> p b l", l=384),
                              in_=_padview(cur, ob0, ob0 + 12))
        oap = ten["out_e"].ap()
        dma(out=oap[:, 24 * hf + 0:24 * hf + 24:2, :],
            in_=o32[0:64, :].rearrange("p (b l) -> p b l", l=384))
        dma(out=oap[:, 24 * hf + 1:24 * hf + 24:2, :],
            in_=o32[64:128, :].rearrange("p (b l) -> p b l", l=384))
    WP.release()
    ctx.close()


def _bcast_b1(ab1):
    base = ab1[:, 1:2]
    return bass.AP(tensor=base.tensor, offset=base.offset,
                   ap=[base.ap[0], [0, NBLK]])


# ======================= host side =======================

_NC = None


def _get_nc():
    global _NC
    if _NC is None:
        _NC = build()
    return _NC


def _prep(x_1d, x_2d, W1, g1, b1, W2, g2, b2, W3, g3, b3,
          res_w, res_b, res_g, res_beta):
    n16 = lambda a: np.ascontiguousarray(np.asarray(a, np.float32).astype(np.float16))
    n32 = lambda a: np.ascontiguousarray(np.asarray(a, np.float32))

    x1 = np.zeros((896, L), np.float16)
    x1[:D1] = n16(x_1d[0])
    x2f = n16(x_2d[0])                      # [210, 384, 384]

    W1 = np.asarray(W1, np.float32)
    w1a = np.zeros((7, 128, 64), np.float16)
    w1c = np.zeros((7, 128, 128), np.float16)
    for a in range(7):
        r0, r1 = a * 128, min((a + 1) * 128, D1)
        w1a[a, :r1 - r0, :] = W1[:, :D1][:, r0:r1].T.astype(np.float16)
        wb = W1[:, D1:][:, r0:r1].T.astype(np.float16)
        w1c[a, :r1 - r0, 0:64] = wb
        w1c[a, :r1 - r0, 64:128] = wb
    W2 = np.asarray(W2, np.float32)
    w2a = np.ascontiguousarray(W2[:, 0:128].T.astype(np.float16))
    w2b = np.zeros((128, 64), np.float16)
    w2b[0:82] = W2[:, 128:210].T.astype(np.float16)
    W3 = np.asarray(W3, np.float32)
    e2 = np.eye(2, dtype=np.float32)
    w3 = np.stack([np.kron(e2, W3[:, :CH].T), np.kron(e2, W3[:, CH:].T)])
    w3 = w3.astype(np.float16)
    wf = np.zeros((10, 9, 128, 128), np.float16)
    wh = np.zeros((10, 6, 128, 64), np.float16)
    res_w = np.asarray(res_w, np.float32)
    for q in range(10):
        w = res_w[q // 2, q % 2]            # [O, I, 3, 3]
        dq = DILS[q]
        if dq != 1:
            for ki in range(3):
                for kj in range(3):
                    tT = w[:, :, ki, kj].T.astype(np.float16)   # [I, O]
                    wf[q, ki * 3 + kj] = np.kron(e2, tT)
        else:
            for kj in range(3):
                # fused: diag = ki=1 (both phases); offdiags carry the
                # same-column-offset halves of ki=0 (p1-out) / ki=2 (p0-out)
                t1 = w[:, :, 1, kj].T.astype(np.float16)
                t0 = w[:, :, 0, kj].T.astype(np.float16)
                t2 = w[:, :, 2, kj].T.astype(np.float16)
                m = np.kron(e2, t1)
                m[0:64, 64:128] = t0      # out_p1 += t0.T' @ in_p0
                m[64:128, 0:64] = t2      # out_p0 += t2.T' @ in_p1
                wf[q, 3 + kj] = m
                wh[q, kj, 64:128] = t0        # leftover: p0-out from phase1 @ -392
                wh[q, 3 + kj, 0:64] = t2      # leftover: p1-out from phase0 @ +392
    eyeS = np.zeros((128, 64), np.float32)
    eyeD = np.zeros((64, 128), np.float32)
    for m in range(64):
        eyeS[m, m] = eyeS[m + 64, m] = 1.0 / 16.0
        eyeD[m, m] = eyeD[m, m + 64] = 1.0
    gpair = n32(np.stack([g1, g2, g3], 1))
    bpair = n32(np.stack([b1, b2, b3], 1))
    gresv = n32(np.asarray(res_g, np.float32).reshape(10, CH).T)
    bresv = n32(np.asarray(res_beta, np.float32).reshape(10, CH).T)

    common = dict(x1=x1, w1a=w1a, w1c=w1c, w2a=w2a, w2b=w2b, w3=w3, wf=wf,
                  wh=wh, eyeS=eyeS, eyeD=eyeD, gpair=gpair, bpair=bpair,
                  gres=gresv, bres=bresv)
    in_maps = []
    for c in range(NCOR):
        r0 = 48 * c - 24
        x2s = np.zeros((D2, 96, L), np.float16)
        lo, hi = max(0, r0), min(L, r0 + 96)
        x2s[:, lo - r0:hi - r0, :] = x2f[:, lo:hi, :]
        x1l = np.zeros((896, 96), np.float16)
        x1l[:, lo - r0:hi - r0] = x1[:, lo:hi]
        msk = np.zeros((128, NBLK), np.float32)
        for p in range(2):
            for b in range(NBLK):
                r = r0 + 2 * b + p
                if 0 <= r < L:
                    msk[64 * p:64 * (p + 1), b] = 1.0
        in_maps.append(dict(common, x2=x2s, x1loc=x1l, mask=msk))
    return in_maps


def _run(inputs, trace=False):
    nc = _get_nc()
    in_maps = _prep(**inputs)
    res = run_bass_kernel_spmd(nc, in_maps, core_ids=list(range(NCOR)),
                               trace=trace)
    out = np.empty((1, CH, L, L), np.float32)
    for c in range(NCOR):
        out[0, :, 48 * c:48 * (c + 1), :] = res.results[c]["out"]
    return out, res


def kernel(**inputs):
    out, _ = _run(inputs, trace=False)
    return out


# revision 12
# speedup vs baseline: 1.3662x; 1.1315x over previous
"""nn_Intra_ResNet on 8 TRN2 NeuronCores (Bass/Tile, SPMD).

Row-sharded 8-way (48 rows/core) with 20-row halo recompute (no halo
exchange). Activations live in SBUF phase-packed: partition = ch + 64*(row
parity), column = block*392 + 4 + j  (a "block" is a pair of image rows,
392 = 384 + 2*4 zero pad columns). Every 3x3 dilated conv tap is then a
K=128/M=128 fp16 matmul at a column offset (even-d taps and d=1 center
row), or a pair of concurrent 64x64 quadrant matmuls via tile_position
(d=1 phase-flip taps). InstanceNorm: per-partition bn_stats + one [128,2]
fp32 AllReduce per norm (13 total), then a fused scale/bias (+mask) +
LeakyReLU applied with 3 big strided ops (gpsimd z-pass, scalar Lrelu,
vector residual-add).
"""
import sys

for _p in ("/opt/trn_rl_repo",):
    if _p not in sys.path:
        sys.path.insert(0, _p)

import numpy as np

import concourse.bass as bass
import concourse.tile as tile
from concourse import bacc, mybir
from concourse.bass_utils import run_bass_kernel_spmd

f16 = mybir.dt.float16
f32 = mybir.dt.float32
AF = mybir.ActivationFunctionType
OP = mybir.AluOpType

NCOR = 8
L = 384
CH = 64
D1 = 788
D2 = 210
EPS = 1e-5
DILS = [1, 1, 2, 2, 4, 4, 2, 2, 1, 1]
OWN = 48          # rows per core
NBLK = 48         # buffer blocks (96 rows incl 2 guard blocks/side)
WPAD = 392        # padded row width (4 | 384 | 4)
PADL = 4
RG = [list(range(NCOR))]

# halo rows needed before conv q; conv q computes local rows [24-H[q+1], 72+H[q+1])
H = [sum(DILS[i:]) for i in range(11)]          # [20,19,18,16,14,10,6,4,2,1,0]
B0 = [(24 - H[q + 1]) // 2 for q in range(10)]  # [2,3,4,5,7,9,10,11,11,12]
B1 = [(72 + H[q + 1] + 1) // 2 for q in range(10)]
PB0, PB1 = 2, 46   # pair-stage computed blocks
OB0, OB1 = 12, 36  # owned blocks


def _padview(buf, b0, b1):
    """[128, b1-b0, 384] view of padded-layout buffer (data cols only)."""
    base = buf[:, 0:1]
    return bass.AP(tensor=base.tensor, offset=base.offset + b0 * WPAD + PADL,
                   ap=[base.ap[0], [WPAD, b1 - b0], [1, 384]])


def _bcast(t2d, b0, b1):
    """[128, b1-b0, 384] broadcast view of a [128, NBLK] per-block tile."""
    base = t2d[:, b0:b1]
    return bass.AP(tensor=base.tensor, offset=base.offset,
                   ap=[base.ap[0], base.ap[1], [0, 384]])


def _rawview(raw, b0, b1):
    return raw[:, b0 * 384:b1 * 384].rearrange("p (b l) -> p b l", l=384)


def build():
    nc = bacc.Bacc("TRN2", target_bir_lowering=False, debug=False,
                   num_devices=NCOR)
    ein = dict(kind="ExternalInput")
    x1_e = nc.dram_tensor("x1", [896, L], f16, **ein)
    x1l_e = nc.dram_tensor("x1loc", [896, 96], f16, **ein)
    x2_e = nc.dram_tensor("x2", [D2, 96, L], f16, **ein)
    mask_e = nc.dram_tensor("mask", [128, NBLK], f32, **ein)
    w1a_e = nc.dram_tensor("w1a", [7, 128, 64], f16, **ein)
    w1c_e = nc.dram_tensor("w1c", [7, 128, 128], f16, **ein)
    w2a_e = nc.dram_tensor("w2a", [128, 64], f16, **ein)
    w2b_e = nc.dram_tensor("w2b", [128, 64], f16, **ein)
    w3_e = nc.dram_tensor("w3", [2, 128, 128], f16, **ein)
    wf_e = nc.dram_tensor("wf", [10, 9, 128, 128], f16, **ein)
    wh_e = nc.dram_tensor("wh", [10, 6, 128, 64], f16, **ein)
    eyeS_e = nc.dram_tensor("eyeS", [128, 64], f32, **ein)
    eyeD_e = nc.dram_tensor("eyeD", [64, 128], f32, **ein)
    gp_e = nc.dram_tensor("gpair", [64, 3], f32, **ein)
    bp_e = nc.dram_tensor("bpair", [64, 3], f32, **ein)
    gr_e = nc.dram_tensor("gres", [64, 10], f32, **ein)
    br_e = nc.dram_tensor("bres", [64, 10], f32, **ein)
    out_e = nc.dram_tensor("out", [CH, OWN, L], f32, kind="ExternalOutput")

    with tile.TileContext(nc) as tc:
        _body(nc, tc, locals())
    nc.compile()
    return nc


def _body(nc, tc, ten):
    from contextlib import ExitStack
    ctx = ExitStack()
    P = ctx.enter_context(tc.tile_pool(name="persist", bufs=1))
    ST = ctx.enter_context(tc.tile_pool(name="stats", bufs=2))
    PS = ctx.enter_context(tc.tile_pool(name="psum", bufs=6, space="PSUM"))
    PSS = ctx.enter_context(tc.tile_pool(name="psmall", bufs=1, space="PSUM"))
    DR = ctx.enter_context(tc.tile_pool(name="dram", bufs=2, space="DRAM"))

    dma = nc.sync.dma_start

    bufA = P.tile([128, NBLK * WPAD], f16)
    bufB = P.tile([128, NBLK * WPAD], f16)
    bufC = P.tile([128, NBLK * WPAD], f16)
    raw = P.tile([128, NBLK * 384], f16)
    mask = P.tile([128, NBLK], f32)
    eyeS = P.tile([128, 64], f32)
    eyeD = P.tile([64, 128], f32)
    gpair = P.tile([64, 3], f32)
    bpair = P.tile([64, 3], f32)
    gres = P.tile([64, 10], f32)
    bres = P.tile([64, 10], f32)
    w3sb = P.tile([128, 2, 128], f16)
    eps = P.tile([128, 1], f32)

    dma(out=mask[:], in_=ten["mask_e"].ap())
    dma(out=eyeS[:], in_=ten["eyeS_e"].ap())
    dma(out=eyeD[:], in_=ten["eyeD_e"].ap())
    dma(out=gpair[:], in_=ten["gp_e"].ap())
    dma(out=bpair[:], in_=ten["bp_e"].ap())
    dma(out=gres[:], in_=ten["gr_e"].ap())
    dma(out=bres[:], in_=ten["br_e"].ap())
    dma(out=w3sb[:], in_=ten["w3_e"].ap().rearrange("t k m -> k t m"))
    nc.vector.memset(eps[:], float(EPS))

    nc.vector.memset(bufA[:], 0.0)
    nc.gpsimd.memset(bufB[:], 0.0)
    nc.vector.memset(bufC[:], 0.0)

    def all_reduce(ms, width):
        arin = DR.tile([128, width], f32, tag="arin")
        arout = DR.tile([128, width], f32, tag="arout", addr_space="Shared")
        dma(out=arin[:], in_=ms[:, 0:width])
        nc.gpsimd.collective_compute(
            "AllReduce", OP.add, replica_groups=RG,
            ins=[arin[:].opt()], outs=[arout[:].opt()])
        armv = ST.tile([128, width], f32, tag="armv")
        dma(out=armv[:, 0:width], in_=arout[:])
        return armv

    def stats_owned():
        """bn_stats over owned blocks of raw -> [128,2] sbuf [mean, msq]."""
        st6 = ST.tile([128, 18, 6], f32, tag="st6")
        for i in range(18):
            nc.vector.bn_stats(out=st6[:, i, :],
                               in_=raw[:, 4608 + 512 * i:4608 + 512 * (i + 1)])
        mv = ST.tile([128, 2], f32, tag="mv")
        nc.vector.bn_aggr(out=mv[:], in_=st6[:])
        ms = ST.tile([128, 2], f32, tag="ms")
        nc.vector.tensor_copy(out=ms[:, 0:1], in_=mv[:, 0:1])
        nc.vector.scalar_tensor_tensor(out=ms[:, 1:2], in0=mv[:, 0:1],
                                       scalar=mv[:, 0:1], in1=mv[:, 1:2],
                                       op0=OP.mult, op1=OP.add)
        return ms

    def stats_to_ab(armv, g64, b64):
        """armv [128,2] (sum over 8 cores of per-partition [mean,msq]) ->
        ab [128,2] f32: col0 A = g*rstd, col1 B' = gmean*A - beta."""
        pm = PSS.tile([64, 2], f32, tag="pm")
        nc.tensor.matmul(pm[:], eyeS[:, :], armv[:, 0:2], start=True, stop=True)
        pmv = ST.tile([64, 2], f32, tag="pmv")
        nc.scalar.activation(out=pmv[:], in_=pm[:], func=AF.Copy)
        ab64 = ST.tile([64, 2], f32, tag="ab64")
        varm = ST.tile([64, 1], f32, tag="varm")
        nc.vector.scalar_tensor_tensor(out=varm[:], in0=pmv[:, 0:1],
                                       scalar=pmv[:, 0:1], in1=pmv[:, 1:2],
                                       op0=OP.mult, op1=OP.subtract)  # -var
        nc.scalar.activation(out=varm[:], in_=varm[:], func=AF.Sqrt,
                             scale=-1.0, bias=eps[0:64, 0:1])
        nc.vector.reciprocal(out=varm[:], in_=varm[:])                # rstd
        nc.vector.tensor_tensor(out=ab64[:, 0:1], in0=g64, in1=varm[:],
                                op=OP.mult)                           # A
        negA = ST.tile([64, 1], f32, tag="negA")
        nc.vector.tensor_scalar(out=negA[:], in0=ab64[:, 0:1], scalar1=-1.0,
                                scalar2=None, op0=OP.mult)
        nc.vector.scalar_tensor_tensor(out=ab64[:, 1:2], in0=pmv[:, 0:1],
                                       scalar=negA[:], in1=b64,
                                       op0=OP.mult, op1=OP.add)  # B''=b-gmean*A
        pd = PSS.tile([128, 2], f32, tag="pd")
        nc.tensor.matmul(pd[:], eyeD[:, :], ab64[:, :], start=True, stop=True)
        ab = ST.tile([128, 2], f32, tag="ab")
        nc.vector.tensor_copy(out=ab[:], in_=pd[:])
        return ab

    def norm_tail(ab, tgt, b0, b1, res=None):
        """Per-block fused z+Lrelu on scalar: out = Lrelu(A*raw + B''*m_b);
        residual added per 4-block chunk on vector. Owned blocks first so
        the next conv's inner-owned matmuls start earliest."""
        Bm = ST.tile([128, NBLK], f32, tag="Bm")
        nc.vector.tensor_scalar(out=Bm[:], in0=mask[:], scalar1=ab[:, 1:2],
                                scalar2=None, op0=OP.mult)
        order = list(range(OB0, OB1)) + list(range(b0, OB0)) + list(range(OB1, b1))
        for i in range(0, len(order), 4):
            blks = order[i:i + 4]
            for b in blks:
                nc.scalar.activation(out=_padview(tgt, b, b + 1),
                                     in_=_rawview(raw, b, b + 1),
                                     func=AF.Lrelu, alpha=0.01,
                                     scale=ab[:, 0:1], bias=Bm[:, b:b + 1])
            if res is not None:
                c0, c1 = blks[0], blks[-1] + 1
                if c1 - c0 == len(blks):  # contiguous run
                    tv = _padview(tgt, c0, c1)
                    nc.vector.tensor_tensor(out=tv, in0=tv,
                                            in1=_padview(res, c0, c1), op=OP.add)
                else:
                    for b in blks:
                        tv = _padview(tgt, b, b + 1)
                        nc.vector.tensor_tensor(out=tv, in0=tv,
                                                in1=_padview(res, b, b + 1),
                                                op=OP.add)

    def evict(pp, b, n=392, lo=PADL):
        """psum block -> raw, masked per block (DVE tensor_scalar)."""
        nc.vector.tensor_scalar(out=raw[:, b * 384:(b + 1) * 384],
                                in0=pp[:, lo:lo + 384],
                                scalar1=mask[:, b:b + 1], scalar2=None,
                                op0=OP.mult)

    # ---------------- x2 stats pass (owned rows) + AR ----------------
    PAIRP = tc.alloc_tile_pool(name="pairp", bufs=2)
    st6a = PAIRP.tile([128, 36, 6], f32, bufs=1)
    st6b = PAIRP.tile([82, 36, 6], f32, bufs=1)
    qs = [nc.sync, nc.gpsimd, nc.scalar]
    for t in range(12):
        xa = PAIRP.tile([128, 4 * 384], f16, tag="xa", name=f"xas{t}")
        xb = PAIRP.tile([82, 4 * 384], f16, tag="xb", name=f"xbs{t}")
        qs[t % 3].dma_start(out=xa[:].rearrange("p (r l) -> p r l", l=L),
                            in_=ten["x2_e"].ap()[0:128, 24 + 4 * t:28 + 4 * t, :])
        qs[(t + 1) % 3].dma_start(out=xb[:].rearrange("p (r l) -> p r l", l=L),
                                  in_=ten["x2_e"].ap()[128:210, 24 + 4 * t:28 + 4 * t, :])
        for i in range(3):
            nc.vector.bn_stats(out=st6a[:, 3 * t + i, :],
                               in_=xa[:, 512 * i:512 * (i + 1)])
            nc.vector.bn_stats(out=st6b[:, 3 * t + i, :],
                               in_=xb[:, 512 * i:512 * (i + 1)])
    mvx = ST.tile([128, 4], f32, tag="mvx")
    nc.vector.memset(mvx[:], 0.0)
    mva = ST.tile([128, 2], f32, tag="mv")
    nc.vector.bn_aggr(out=mva[:], in_=st6a[:])
    mvb = ST.tile([82, 2], f32, tag="mvb")
    nc.vector.bn_aggr(out=mvb[:], in_=st6b[:])
    nc.vector.tensor_copy(out=mvx[:, 0:1], in_=mva[:, 0:1])
    nc.vector.scalar_tensor_tensor(out=mvx[:, 1:2], in0=mva[:, 0:1],
                                   scalar=mva[:, 0:1], in1=mva[:, 1:2],
                                   op0=OP.mult, op1=OP.add)
    nc.vector.tensor_copy(out=mvx[0:82, 2:3], in_=mvb[:, 0:1])
    nc.vector.scalar_tensor_tensor(out=mvx[0:82, 3:4], in0=mvb[:, 0:1],
                                   scalar=mvb[:, 0:1], in1=mvb[:, 1:2],
                                   op0=OP.mult, op1=OP.add)
    armx = all_reduce(mvx, 4)   # AR#1 (in flight during pair1)

    # ---------------- x1 norm + pair1 into bufA ----------------
    x1sb = PAIRP.tile([128, 7, L], f16, bufs=1)
    x1n = PAIRP.tile([128, 7, L], f16, bufs=1)
    x1ls = PAIRP.tile([128, 7, 96], f16, bufs=1)
    x1nl = PAIRP.tile([128, 7, 96], f16, bufs=1)
    w1a = PAIRP.tile([128, 7, 64], f16, bufs=1)
    w1c = PAIRP.tile([128, 7, 128], f16, bufs=1)
    dma(out=x1sb[:], in_=ten["x1_e"].ap().rearrange("(a p) l -> p a l", p=128))
    dma(out=x1ls[:], in_=ten["x1l_e"].ap().rearrange("(a p) l -> p a l", p=128))
    dma(out=w1a[:], in_=ten["w1a_e"].ap().rearrange("a k m -> k a m"))
    dma(out=w1c[:], in_=ten["w1c_e"].ap().rearrange("a k m -> k a m"))
    st1 = ST.tile([128, 6], f32, tag="st1")
    mv1 = ST.tile([128, 7, 2], f32, tag="mv1")
    for j in range(7):
        nc.vector.bn_stats(out=st1[:], in_=x1sb[:, j, :])
        nc.vector.bn_aggr(out=mv1[:, j, :], in_=st1[:])
    rst1 = ST.tile([128, 7], f32, tag="rst1")
    nc.scalar.activation(out=rst1[:], in_=mv1[:, :, 1], func=AF.Sqrt,
                         bias=eps[:, 0:1])
    nc.vector.reciprocal(out=rst1[:], in_=rst1[:])
    for j in range(7):
        nc.vector.tensor_scalar(out=x1n[:, j, :], in0=x1sb[:, j, :],
                                scalar1=mv1[:, j, 0:1], scalar2=rst1[:, j:j + 1],
                                op0=OP.subtract, op1=OP.mult)
        nc.vector.tensor_scalar(out=x1nl[:, j, :], in0=x1ls[:, j, :],
                                scalar1=mv1[:, j, 0:1], scalar2=rst1[:, j:j + 1],
                                op0=OP.subtract, op1=OP.mult)

    rowg_p = PS.tile([64, L], f32, tag="pp")
    rowl_p = PS.tile([64, 96], f32, tag="pp")
    colD_p = PS.tile([128, L], f32, tag="pp")
    for j in range(7):
        kw = dict(start=(j == 0), stop=(j == 6))
        nc.tensor.matmul(rowg_p[:], w1a[:, j, :], x1n[:, j, :], **kw)
        nc.tensor.matmul(rowl_p[:], w1a[:, j, :], x1nl[:, j, :], **kw)
        nc.tensor.matmul(colD_p[:], w1c[:, j, :], x1n[:, j, :], **kw)
    colD = PAIRP.tile([128, L], f16, bufs=1)
    nc.scalar.activation(out=colD[:], in_=colD_p[:], func=AF.Copy)
    rl32 = PAIRP.tile([64, 96], f32, bufs=1)
    nc.scalar.activation(out=rl32[:], in_=rowl_p[:], func=AF.Copy)
    rowv = PAIRP.tile([128, NBLK], f32, bufs=1)
    dma(out=rowv[0:64, :], in_=rl32[:, 0::2])
    dma(out=rowv[64:128, :], in_=rl32[:, 1::2])

    # pair1 stats (exact, local): mean/var = row stats + col stats
    strc = ST.tile([64, 6], f32, tag="strc")
    mvp1 = ST.tile([64, 2], f32, tag="mvp1")
    mvc1 = ST.tile([64, 2], f32, tag="mvc1")
    nc.vector.bn_stats(out=strc[:], in_=rowg_p[:, :])
    nc.vector.bn_aggr(out=mvp1[:], in_=strc[:])
    nc.vector.bn_stats(out=strc[:], in_=colD[0:64, :])
    nc.vector.bn_aggr(out=mvc1[:], in_=strc[:])
    nc.vector.tensor_tensor(out=mvp1[:], in0=mvp1[:], in1=mvc1[:], op=OP.add)
    # A1 = g1*rstd, B1' = mean*A1 - b1  (no AR, no phase combine needed)
    ab64 = ST.tile([64, 2], f32, tag="ab64")
    varm = ST.tile([64, 1], f32, tag="varm")
    nc.scalar.activation(out=varm[:], in_=mvp1[:, 1:2], func=AF.Sqrt,
                         bias=eps[0:64, 0:1])
    nc.vector.reciprocal(out=varm[:], in_=varm[:])
    nc.vector.tensor_tensor(out=ab64[:, 0:1], in0=gpair[:, 0:1], in1=varm[:],
                            op=OP.mult)
    negA1 = ST.tile([64, 1], f32, tag="negA")
    nc.vector.tensor_scalar(out=negA1[:], in0=ab64[:, 0:1], scalar1=-1.0,
                            scalar2=None, op0=OP.mult)
    nc.vector.scalar_tensor_tensor(out=ab64[:, 1:2], in0=mvp1[:, 0:1],
                                   scalar=negA1[:], in1=bpair[:, 0:1],
                                   op0=OP.mult, op1=OP.add)
    pd1 = PSS.tile([128, 2], f32, tag="pd")
    nc.tensor.matmul(pd1[:], eyeD[:, :], ab64[:, :], start=True, stop=True)
    ab1 = ST.tile([128, 2], f32, tag="ab")
    nc.vector.tensor_copy(out=ab1[:], in_=pd1[:])
    # Rv = A1*rowv - B1' ; p1 = lrelu(colD*A1 + Rv)
    Rv = ST.tile([128, NBLK], f32, tag="Bm")
    nc.vector.scalar_tensor_tensor(out=Rv[:], in0=rowv[:], scalar=ab1[:, 0:1],
                                   in1=_bcast_b1(ab1), op0=OP.mult,
                                   op1=OP.add)
    p1v = _padview(bufA, PB0, PB1)
    colD_bc = bass.AP(tensor=colD[:, :].tensor, offset=colD[:, :].offset,
                      ap=[colD[:, :].ap[0], [0, PB1 - PB0], [1, 384]])
    nc.vector.scalar_tensor_tensor(out=p1v, in0=colD_bc, scalar=ab1[:, 0:1],
                                   in1=_bcast(Rv, PB0, PB1),
                                   op0=OP.mult, op1=OP.add)
    nc.scalar.activation(out=p1v, in_=p1v, func=AF.Lrelu, alpha=0.01)

    # ---------------- pair2: scale W2 by rstd(x2), matmul pass ----------------
    w2a = PAIRP.tile([128, 64], f16, bufs=1)
    w2b = PAIRP.tile([128, 64], f16, bufs=1)
    dma(out=w2a[:], in_=ten["w2a_e"].ap())
    dma(out=w2b[:], in_=ten["w2b_e"].ap())
    # rstd for both groups from armx [128,4] = sum over cores [mA,qA,mB,qB]
    mg = ST.tile([128, 2], f32, tag="mg")
    vg = ST.tile([128, 2], f32, tag="vg")
    nc.vector.tensor_scalar(out=mg[:], in0=armx[:, 0::2], scalar1=0.125,
                            scalar2=None, op0=OP.mult)
    nc.vector.tensor_scalar(out=vg[:], in0=armx[:, 1::2], scalar1=0.125,
                            scalar2=None, op0=OP.mult)
    nc.vector.scalar_tensor_tensor(out=vg[:, 0:1], in0=mg[:, 0:1],
                                   scalar=mg[:, 0:1], in1=vg[:, 0:1],
                                   op0=OP.mult, op1=OP.subtract)  # m^2-q = -var
    nc.vector.scalar_tensor_tensor(out=vg[:, 1:2], in0=mg[:, 1:2],
                                   scalar=mg[:, 1:2], in1=vg[:, 1:2],
                                   op0=OP.mult, op1=OP.subtract)
    nc.scalar.activation(out=vg[:], in_=vg[:], func=AF.Sqrt, scale=-1.0,
                         bias=eps[:, 0:1])
    nc.vector.reciprocal(out=vg[:], in_=vg[:])   # rstd [128,2]
    w2as = PAIRP.tile([128, 64], f16, bufs=1)
    w2bs = PAIRP.tile([128, 64], f16, bufs=1)
    nc.vector.tensor_scalar(out=w2as[:], in0=w2a[:], scalar1=vg[:, 0:1],
                            scalar2=None, op0=OP.mult)
    nc.vector.tensor_scalar(out=w2bs[:], in0=w2b[:], scalar1=vg[:, 1:2],
                            scalar2=None, op0=OP.mult)

    for ci in list(range(5, 17)) + list(range(0, 5)) + list(range(17, 22)):
        xa = PAIRP.tile([128, 4 * 384], f16, tag="xa", name=f"xam{ci}")
        xb = PAIRP.tile([82, 4 * 384], f16, tag="xb", name=f"xbm{ci}")
        qs[ci % 3].dma_start(out=xa[:].rearrange("p (r l) -> p r l", l=L),
                             in_=ten["x2_e"].ap()[0:128, 4 + 4 * ci:8 + 4 * ci, :])
        qs[(ci + 1) % 3].dma_start(out=xb[:].rearrange("p (r l) -> p r l", l=L),
                                   in_=ten["x2_e"].ap()[128:210, 4 + 4 * ci:8 + 4 * ci, :])
        for t in range(2):
            b = PB0 + 2 * ci + t
            pp = PS.tile([128, 392], f32, tag="pp")
            er, od = 2 * t, 2 * t + 1
            nc.tensor.matmul(pp[0:64, 0:384], w2as[:, :],
                             xa[:, er * 384:(er + 1) * 384], start=True, stop=False)
            nc.tensor.matmul(pp[0:64, 0:384], w2bs[0:82, :],
                             xb[0:82, er * 384:(er + 1) * 384], start=False, stop=True)
            nc.tensor.matmul(pp[64:128, 0:384], w2as[:, :],
                             xa[:, od * 384:(od + 1) * 384], start=True, stop=False,
                             tile_position=(0, 64))
            nc.tensor.matmul(pp[64:128, 0:384], w2bs[0:82, :],
                             xb[0:82, od * 384:(od + 1) * 384], start=False, stop=True,
                             tile_position=(0, 64))
            evict(pp, b, lo=0)
        if ci == 16:  # owned blocks [12,36) done -> kick stats+AR early
            ms2 = stats_owned()
            arm2 = all_reduce(ms2, 2)
            ab2 = stats_to_ab(arm2, gpair[:, 1:2], bpair[:, 1:2])
    norm_tail(ab2, bufB, PB0, PB1)
    PAIRP.release()
    WP = tc.alloc_tile_pool(name="respool", bufs=2)

    # ---------------- pair3 into bufC ----------------
    def p3_group(g0, g1):
        pps = []
        for b in range(g0, g1):
            pp = PS.tile([128, 392], f32, tag="pp", name=f"pp3_{b}")
            pps.append(pp)
            nc.tensor.matmul(pp[:], w3sb[:, 0, :],
                             bufA[:, b * WPAD:(b + 1) * WPAD], start=True, stop=False)
            nc.tensor.matmul(pp[:], w3sb[:, 1, :],
                             bufB[:, b * WPAD:(b + 1) * WPAD], start=False, stop=True)
        for i, b in enumerate(range(g0, g1)):
            evict(pps[i], b)
    for g0 in range(OB0, OB1, 6):
        p3_group(g0, min(g0 + 6, OB1))
    ms3 = stats_owned()
    arm3 = all_reduce(ms3, 2)
    ab3 = stats_to_ab(arm3, gpair[:, 2:3], bpair[:, 2:3])
    for g0 in range(PB0, OB0, 6):
        p3_group(g0, min(g0 + 6, OB0))
    for g0 in range(OB1, PB1, 6):
        p3_group(g0, min(g0 + 6, PB1))
    norm_tail(ab3, bufC, PB0, PB1)

    # ---------------- ResNet: 5 blocks x 2 convs ----------------
    cur, tmp, nxt = bufC, bufA, bufB
    for q in range(10):
        d = DILS[q]
        b0, b1 = B0[q], B1[q]
        IN = cur if q % 2 == 0 else tmp
        TGT = tmp if q % 2 == 0 else nxt
        wf = WP.tile([128, 9, 128], f16, tag="wf")
        dma(out=wf[:], in_=ten["wf_e"].ap()[q].rearrange("t k m -> k t m"))
        if d == 1:
            wh = WP.tile([128, 6, 64], f16, tag="wh")
            dma(out=wh[:], in_=ten["wh_e"].ap()[q].rearrange("t k m -> k t m"))

        # taps: list of (kind, lhsT-getter, rhs partition range, delta, tilepos, outslice)
        taps = []
        if d != 1:
            for ki in range(3):
                for kj in range(3):
                    dd = (ki - 1) * (d // 2) * WPAD + (kj - 1) * d
                    taps.append(("f", ki * 3 + kj, dd))
        else:
            for kj in range(3):
                taps.append(("f", 3 + kj, kj - 1))
            for kj in range(3):
                # leftover phase-flip halves: p0-out reads phase1 @ -392,
                # p1-out reads phase0 @ +392 (concurrent quadrant pair)
                taps.append(("h", (kj, 3 + kj),
                             (-WPAD + kj - 1, WPAD + kj - 1)))
        ntap = len(taps)

        def do_group(g0, g1):
            pps = {b: PS.tile([128, 392], f32, tag="pp", name=f"pp_{q}_{b}")
                   for b in range(g0, g1)}
            for it, (kind, t, dd) in enumerate(taps):
                first, last = it == 0, it == ntap - 1
                for b in range(g0, g1):
                    pp = pps[b]
                    base = b * WPAD
                    if kind == "f":
                        nc.tensor.matmul(pp[:], wf[:, t, :],
                                         IN[:, base + dd:base + dd + WPAD],
                                         start=first, stop=False,
                                         skip_group_check=True)
                    else:
                        t0, t1 = t
                        d0, d1_ = dd
                        nc.tensor.matmul(pp[0:64, :], wh[64:128, t0, :],
                                         IN[64:128, base + d0:base + d0 + WPAD],
                                         start=False, stop=False,
                                         tile_position=(64, 0),
                                         skip_group_check=True)
                        nc.tensor.matmul(pp[64:128, :], wh[0:64, t1, :],
                                         IN[0:64, base + d1_:base + d1_ + WPAD],
                                         start=False, stop=last,
                                         tile_position=(0, 64),
                                         skip_group_check=True)
            for b in range(g0, g1):
                evict(pps[b], b)

        # inner-owned first (inputs need only the previous conv's owned
        # normalize chunks), then outer-owned, then stats+AR, then halo
        # groups (hide the AR), then normalize.
        for g0 in range(OB0 + 2, OB1 - 2, 6):
            do_group(g0, min(g0 + 6, OB1 - 2))
        do_group(OB0, OB0 + 2)
        do_group(OB1 - 2, OB1)
        ms = stats_owned()
        arm = all_reduce(ms, 2)
        ab = stats_to_ab(arm, gres[:, q:q + 1], bres[:, q:q + 1])
        for g0 in range(b0, OB0, 6):
            do_group(g0, min(g0 + 6, OB0))
        for g0 in range(OB1, b1, 6):
            do_group(g0, min(g0 + 6, b1))
        norm_tail(ab, TGT, b0, b1, res=(cur if q % 2 == 1 else None))
        if q % 2 == 1:
            cur, nxt = nxt, cur

    # ---------------- output: owned blocks, fp32, de-phase ----------------
    for hf in range(2):
        ob0 = OB0 + 12 * hf
        o32 = WP.tile([128, 12 * 384], f32, tag="o32", bufs=1, name=f"o32_{hf}")
        nc.vector.tensor_copy(out=o32[:].rearrange("p (b l) -> p b l", l=384),
                              in_=_padview(cur, ob0, ob0 + 12))
        oap = ten["out_e"].ap()
        dma(out=oap[:, 24 * hf + 0:24 * hf + 24:2, :],
            in_=o32[0:64, :].rearrange("p (b l) -> p b l", l=384))
        dma(out=oap[:, 24 * hf + 1:24 * hf + 24:2, :],
            in_=o32[64:128, :].rearrange("p (b l) -> p b l", l=384))
    WP.release()
    ctx.close()


def _bcast_b1(ab1):
    base = ab1[:, 1:2]
    return bass.AP(tensor=base.tensor, offset=base.offset,
                   ap=[base.ap[0], [0, NBLK]])


# ======================= host side =======================

_NC = None


def _get_nc():
    global _NC
    if _NC is None:
        _NC = build()
    return _NC


def _prep(x_1d, x_2d, W1, g1, b1, W2, g2, b2, W3, g3, b3,
          res_w, res_b, res_g, res_beta):
    n16 = lambda a: np.ascontiguousarray(np.asarray(a, np.float32).astype(np.float16))
    n32 = lambda a: np.ascontiguousarray(np.asarray(a, np.float32))

    x1 = np.zeros((896, L), np.float16)
    x1[:D1] = n16(x_1d[0])
    x2f = n16(x_2d[0])                      # [210, 384, 384]

    W1 = np.asarray(W1, np.float32)
    w1a = np.zeros((7, 128, 64), np.float16)
    w1c = np.zeros((7, 128, 128), np.float16)
    for a in range(7):
        r0, r1 = a * 128, min((a + 1) * 128, D1)
        w1a[a, :r1 - r0, :] = W1[:, :D1][:, r0:r1].T.astype(np.float16)
        wb = W1[:, D1:][:, r0:r1].T.astype(np.float16)
        w1c[a, :r1 - r0, 0:64] = wb
        w1c[a, :r1 - r0, 64:128] = wb
    W2 = np.asarray(W2, np.float32)
    w2a = np.ascontiguousarray(W2[:, 0:128].T.astype(np.float16))
    w2b = np.zeros((128, 64), np.float16)
    w2b[0:82] = W2[:, 128:210].T.astype(np.float16)
    W3 = np.asarray(W3, np.float32)
    e2 = np.eye(2, dtype=np.float32)
    w3 = np.stack([np.kron(e2, W3[:, :CH].T), np.kron(e2, W3[:, CH:].T)])
    w3 = w3.astype(np.float16)
    wf = np.zeros((10, 9, 128, 128), np.float16)
    wh = np.zeros((10, 6, 128, 64), np.float16)
    res_w = np.asarray(res_w, np.float32)
    for q in range(10):
        w = res_w[q // 2, q % 2]            # [O, I, 3, 3]
        dq = DILS[q]
        if dq != 1:
            for ki in range(3):
                for kj in range(3):
                    tT = w[:, :, ki, kj].T.astype(np.float16)   # [I, O]
                    wf[q, ki * 3 + kj] = np.kron(e2, tT)
        else:
            for kj in range(3):
                # fused: diag = ki=1 (both phases); offdiags carry the
                # same-column-offset halves of ki=0 (p1-out) / ki=2 (p0-out)
                t1 = w[:, :, 1, kj].T.astype(np.float16)
                t0 = w[:, :, 0, kj].T.astype(np.float16)
                t2 = w[:, :, 2, kj].T.astype(np.float16)
                m = np.kron(e2, t1)
                m[0:64, 64:128] = t0      # out_p1 += t0.T' @ in_p0
                m[64:128, 0:64] = t2      # out_p0 += t2.T' @ in_p1
                wf[q, 3 + kj] = m
                wh[q, kj, 64:128] = t0        # leftover: p0-out from phase1 @ -392
                wh[q, 3 + kj, 0:64] = t2      # leftover: p1-out from phase0 @ +392
    eyeS = np.zeros((128, 64), np.float32)
    eyeD = np.zeros((64, 128), np.float32)
    for m in range(64):
        eyeS[m, m] = eyeS[m + 64, m] = 1.0 / 16.0
        eyeD[m, m] = eyeD[m, m + 64] = 1.0
    gpair = n32(np.stack([g1, g2, g3], 1))
    bpair = n32(np.stack([b1, b2, b3], 1))
    gresv = n32(np.asarray(res_g, np.float32).reshape(10, CH).T)
    bresv = n32(np.asarray(res_beta, np.float32).reshape(10, CH).T)

    common = dict(x1=x1, w1a=w1a, w1c=w1c, w2a=w2a, w2b=w2b, w3=w3, wf=wf,
                  wh=wh, eyeS=eyeS, eyeD=eyeD, gpair=gpair, bpair=bpair,
                  gres=gresv, bres=bresv)
    in_maps = []
    for c in range(NCOR):
        r0 = 48 * c - 24
        x2s = np.zeros((D2, 96, L), np.float16)
        lo, hi = max(0, r0), min(L, r0 + 96)
        x2s[:, lo - r0:hi - r0, :] = x2f[:, lo:hi, :]
        x1l = np.zeros((896, 96), np.float16)
        x1l[:, lo - r0:hi - r0] = x1[:, lo:hi]
        msk = np.zeros((128, NBLK), np.float32)
        for p in range(2):
            for b in range(NBLK):
                r = r0 + 2 * b + p
                if 0 <= r < L:
                    msk[64 * p:64 * (p + 1), b] = 1.0
        in_maps.append(dict(common, x2=x2s, x1loc=x1l, mask=msk))
    return in_maps


def _run(inputs, trace=False):
    nc = _get_nc()
    in_maps = _prep(**inputs)
    res = run_bass_kernel_spmd(nc, in_maps, core_ids=list(range(NCOR)),
                               trace=trace)
    out = np.empty((1, CH, L, L), np.float32)
    for c in range(NCOR):
        out[0, :, 48 * c:48 * (c + 1), :] = res.results[c]["out"]
    return out, res


def kernel(**inputs):
    out, _ = _run(inputs, trace=False)
    return out
